# revision 3
# baseline (speedup 1.0000x reference)
"""CrossModalMoELayer Trainium2 Bass kernel.

Sharding: data-parallel over batch B=8 across the 8 NeuronCores (one batch
element per core). Each core runs the full layer for its batch element:
self-attention, cross-attention, gating, and the dense 8-expert MoE.

Attention/gating run in float32r (full-rate fp32 PE mode). The MoE - the
dominant compute - runs in fp8 (e4m3) with DoubleRow perf mode (2 fp8
MACs/cell/cycle, ~2x PE throughput). Expert weights are host-quantized to
e4m3 with a x256 scale (keeps the 0.02-scale weights out of the subnormal
range); the scale is folded back out in the gelu (scale=1/256) and the
router-prob accumulate (probs/256).

Layouts on device:
  feature-major ("fm"): [feat_part=128, feat_chunk, tokens]  - activations
  MoE output accumulates token-major: [tok_part=128, tok_tile, feature] so
  router probs apply as native per-partition scalars and the final store
  needs no transposes.

kernel(**inputs) takes the FULL unsharded inputs (numpy, keyed as in
setup_inputs()) and returns the full (query_tokens, image_tokens) tuple.
"""

import numpy as np
import ml_dtypes

import concourse.bass as bass
import concourse.tile as tile
from concourse import bacc, mybir
from concourse.bass_utils import run_bass_kernel_spmd
from concourse.masks import make_identity

B, T, H, NH, HD, F, E = 8, 256, 1024, 16, 64, 4096, 8
IC = H // 128          # 8 feature chunks of the model dim
FT = F // 128          # 32 feature chunks of the FFN dim
KT1 = IC // 2          # 4 DoubleRow k-tiles for GEMM1 (contraction H)
KT2 = FT // 2          # 16 DoubleRow k-tiles for GEMM2 (contraction F)
NTT = 4                # token tiles of 128 across both streams
T2 = 2 * T             # query tokens + image tokens concatenated
EPS = 1e-5
WS = 256.0             # fp8 weight scale

F32 = mybir.dt.float32
F32R = mybir.dt.float32r
FP8 = mybir.dt.float8e4
E4NP = ml_dtypes.float8_e4m3
AX = mybir.AxisListType
ALU = mybir.AluOpType
AF = mybir.ActivationFunctionType
DR = mybir.MatmulPerfMode.DoubleRow


# ----------------------------------------------------------------------------
# program builder
# ----------------------------------------------------------------------------

def _build_program():
    nc = bacc.Bacc(
        "TRN2",
        target_bir_lowering=False,
        debug=False,
        enable_asserts=False,
        num_devices=8,
    )

    dt = {}

    def din(name, shape, d=F32):
        dt[name] = nc.dram_tensor(name, list(shape), d, kind="ExternalInput").ap()
        return dt[name]

    def dout(name, shape):
        dt[name] = nc.dram_tensor(name, list(shape), F32, kind="ExternalOutput").ap()
        return dt[name]

    # activations (per core)
    din("xq", (128, IC, T), F32R)
    din("xi", (128, IC, T), F32R)
    din("xt", (128, IC, T), F32R)
    # attention weights: [proj, ot, i(128), ic, o(128)]
    din("w_sa", (3, 8, 128, IC, 128), F32R)
    din("b_sa", (128, 3, IC))
    din("w_sao", (8, 128, IC, 128), F32R)
    din("b_sao", (128, IC))
    din("w_ca", (3, 8, 128, IC, 128), F32R)
    din("b_ca", (128, 3, IC))
    din("w_cao", (8, 128, IC, 128), F32R)
    din("b_cao", (128, IC))
    # gates
    din("w_ig1", (128, IC, E), F32R)
    din("w_ig2", (128, IC, E), F32R)
    din("b_ig", (1, E))
    din("w_tg1", (128, IC, E), F32R)
    din("w_tg2", (128, IC, E), F32R)
    din("b_tg", (1, E))
    # layernorms [128, IC]
    for n in ("g_lnq", "b_lnq", "g_lnc", "b_lnc", "g_lnf", "b_lnf"):
        din(n, (128, IC))
    # experts (fp8, x256-scaled, DoubleRow pair layout)
    din("w1", (E, KT1, 128, 2, F), FP8)        # [e, k, i(128), pair, f]
    din("b1", (128, E, FT))
    din("w2", (E, KT2, 128, 2, H), FP8)        # [e, k, f(128), pair, o]
    din("b2", (E, H))
    # outputs token-major [t(128), tt, o]
    dout("oq", (128, 2, H))
    dout("oi", (128, 2, H))

    with tile.TileContext(nc) as tc:
        _trace_kernel(nc, tc, dt)

    nc.compile()
    return nc


def _trace_kernel(nc, tc, dt):
    persist = tc.alloc_tile_pool(name="persist", bufs=1)

    # ---- constants + small params --------------------------------------
    ident = persist.tile([128, 128], F32, tag="ident")
    make_identity(nc, ident)
    ones_f = persist.tile([128, 1], F32, tag="ones_f")
    nc.vector.memset(ones_f, 1.0)
    ones = persist.tile([128, 1], F32R, tag="ones")
    nc.vector.tensor_copy(ones, ones_f)
    identr = persist.tile([128, 128], F32R, tag="identr")
    nc.vector.tensor_copy(identr, ident)
    eps_t = persist.tile([1, 1], F32, tag="eps")
    nc.vector.memset(eps_t, EPS)

    def load(name, shape, d=F32, pool=persist):
        t = pool.tile(list(shape), d, tag=f"ld_{name}")
        nc.sync.dma_start(out=t, in_=dt[name])
        return t

    xi0 = load("xi", (128, IC, T), F32R)
    b_sa = load("b_sa", (128, 3, IC))
    b_sao = load("b_sao", (128, IC))
    b_ca = load("b_ca", (128, 3, IC))
    b_cao = load("b_cao", (128, IC))
    w_ig1 = load("w_ig1", (128, IC, E), F32R)
    w_ig2 = load("w_ig2", (128, IC, E), F32R)
    b_ig = load("b_ig", (1, E))
    w_tg1 = load("w_tg1", (128, IC, E), F32R)
    w_tg2 = load("w_tg2", (128, IC, E), F32R)
    b_tg = load("b_tg", (1, E))
    lnp = {n: load(n, (128, IC)) for n in
           ("g_lnq", "b_lnq", "g_lnc", "b_lnc", "g_lnf", "b_lnf")}
    b1f = load("b1", (128, E, FT))
    b2m = load("b2", (E, H))

    # persistent activations
    q2 = persist.tile([128, IC, T], F32R, tag="q2")          # query after CA
    x8 = persist.tile([128, IC, T2], FP8, tag="x8")          # fp8 [lnf(q2); xi0]
    ptm = persist.tile([128, NTT, E], F32, tag="ptm")        # router probs (tm)
    ps8 = persist.tile([128, NTT, E], F32, tag="ps8")        # probs / 256
    pfm = persist.tile([E, NTT, 128], F32, tag="pfm")        # probs (fm, for b2)
    acc = persist.tile([128, NTT, H], F32, tag="acc")        # MoE accum (tm)

    # ====================================================================
    # phase 1: attention + gating + lnf (own pools, released before MoE)
    # ====================================================================
    aps_mm = tc.alloc_tile_pool(name="aps_mm", bufs=3, space="PSUM")
    aps_tr = tc.alloc_tile_pool(name="aps_tr", bufs=2, space="PSUM")
    aps_pv = tc.alloc_tile_pool(name="aps_pv", bufs=2, space="PSUM")
    aps_sm = tc.alloc_tile_pool(name="aps_sm", bufs=1, space="PSUM")
    awork = tc.alloc_tile_pool(name="awork", bufs=2)
    aw1 = tc.alloc_tile_pool(name="aw1", bufs=1)
    wpool = tc.alloc_tile_pool(name="wpool", bufs=4)

    def ln_fm(dst, src, g, b, dst8=None):
        """dst[:, ic, :] = LN over features of src (fm layout [128, IC, T]).

        If dst8 is given, the normalized result is additionally written to
        dst8 (fp8) with the fp32 intermediate kept in dst.
        """
        ntok = src.shape[2]
        sum_ps = aps_sm.tile([1, ntok], F32, tag="sm")
        for ic in range(IC):
            nc.tensor.matmul(sum_ps, ones, src[:, ic, :],
                             start=(ic == 0), stop=(ic == IC - 1))
        mean = awork.tile([1, ntok], F32, tag="ln_mean")
        nc.scalar.mul(mean, sum_ps, 1.0 / H)
        sumsq_ps = aps_sm.tile([1, ntok], F32, tag="sm")
        for ic in range(IC):
            xsq = awork.tile([128, ntok], F32R, tag="ln_xsq")
            nc.scalar.activation(xsq, src[:, ic, :], AF.Square)
            nc.tensor.matmul(sumsq_ps, ones, xsq,
                             start=(ic == 0), stop=(ic == IC - 1))
        msq = awork.tile([1, ntok], F32, tag="ln_msq")
        nc.vector.tensor_mul(msq, mean, mean)
        var = awork.tile([1, ntok], F32, tag="ln_var")
        nc.vector.scalar_tensor_tensor(var, in0=sumsq_ps, scalar=1.0 / H,
                                       in1=msq, op0=ALU.mult, op1=ALU.subtract)
        std = awork.tile([1, ntok], F32, tag="ln_std")
        nc.scalar.activation(std, var, AF.Sqrt, bias=eps_t)
        rstd = awork.tile([1, ntok], F32, tag="ln_rstd")
        nc.vector.reciprocal(rstd, std)
        negc = awork.tile([1, ntok], F32, tag="ln_negc")
        nc.vector.scalar_tensor_tensor(negc, in0=mean, scalar=-1.0,
                                       in1=rstd, op0=ALU.mult, op1=ALU.mult)
        a_bc = awork.tile([128, ntok], F32, tag="ln_abc")
        nc.gpsimd.partition_broadcast(a_bc, rstd)
        c_bc = awork.tile([128, ntok], F32, tag="ln_cbc")
        nc.gpsimd.partition_broadcast(c_bc, negc)
        for ic in range(IC):
            nc.vector.tensor_mul(dst[:, ic, :], src[:, ic, :], a_bc)
            nc.vector.tensor_add(dst[:, ic, :], dst[:, ic, :], c_bc)
            out_ic = dst[:, ic, :] if dst8 is None else dst8[:, ic, :]
            nc.vector.tensor_scalar(out=out_ic, in0=dst[:, ic, :],
                                    scalar1=g[:, ic:ic + 1], scalar2=b[:, ic:ic + 1],
                                    op0=ALU.mult, op1=ALU.add)

    def proj_fm(dst, src, w_dram_ot, bias, bias_col):
        """dst[:, ot, :] = W @ src + b  (fm in, fm out)."""
        ntok = src.shape[2]
        for ot in range(IC):
            wt = wpool.tile([128, IC, 128], F32R, tag="wsl")
            nc.sync.dma_start(out=wt, in_=w_dram_ot(ot))
            ps = aps_mm.tile([128, ntok], F32, tag="mm")
            for ic in range(IC):
                nc.tensor.matmul(ps, wt[:, ic, :], src[:, ic, :],
                                 start=(ic == 0), stop=(ic == IC - 1))
            nc.scalar.add(dst[:, ot, :], ps, bias[:, bias_col(ot)])

    def attention(new_resid, old_resid, qsrc, kvsrc, w_in, b_in, w_out, b_out):
        """new_resid = old_resid + out_proj(MHA(q=qsrc, kv=kvsrc)); all fm."""
        qf = aw1.tile([128, IC, T], F32R, tag="qf")
        kf = aw1.tile([128, IC, T], F32R, tag="kf")
        vf = aw1.tile([128, IC, T], F32R, tag="vf")
        proj_fm(qf, qsrc, lambda ot: w_in[0, ot], b_in, lambda ot: slice(0 * IC + ot, 0 * IC + ot + 1))
        proj_fm(kf, kvsrc, lambda ot: w_in[1, ot], b_in, lambda ot: slice(1 * IC + ot, 1 * IC + ot + 1))
        proj_fm(vf, kvsrc, lambda ot: w_in[2, ot], b_in, lambda ot: slice(2 * IC + ot, 2 * IC + ot + 1))
        # attention output, token-major: ao_tm[t(128), qt, h*64+d]
        ao_tm = aw1.tile([128, 2, H], F32R, tag="ao_tm")
        for pair in range(NH // 2):
            per_head = []
            for h in (2 * pair, 2 * pair + 1):
                base = (h % 2) * HD
                c = h // 2
                qh = qf[base:base + HD, c, :]
                kh = kf[base:base + HD, c, :]
                vh = vf[base:base + HD, c, :]
                idn = identr[base:base + HD, base:base + HD]
                # vh^T : [T, HD] in two 128-token tiles
                vht = awork.tile([128, 2, HD], F32R, tag="vht",
                                 name=f"vht_{h}")
                for kt in range(2):
                    tp = aps_tr.tile([128, HD], F32R, tag="tr")
                    nc.tensor.transpose(tp, vh[:, kt * 128:(kt + 1) * 128], idn)
                    nc.vector.tensor_copy(vht[:, kt, :], tp)
                attn_t = awork.tile([128, 2, T], F32R, tag="attnT",
                                    name=f"attnT_{h}")
                for qt in range(2):
                    sc = aps_mm.tile([128, T], F32, tag="mm")
                    nc.tensor.matmul(sc, qh[:, qt * 128:(qt + 1) * 128], kh,
                                     start=True, stop=True)
                    nmax = awork.tile([128, 1], F32, tag="nmax")
                    nc.vector.reduce_max(nmax, sc, axis=AX.X, negate=True)
                    nmax2 = awork.tile([128, 1], F32, tag="nmax2")
                    nc.scalar.mul(nmax2, nmax, 0.125)
                    asb = awork.tile([128, T], F32, tag="asb")
                    ssum = awork.tile([128, 1], F32, tag="ssum")
                    nc.scalar.activation(asb, sc, AF.Exp, bias=nmax2, scale=0.125,
                                         accum_out=ssum)
                    rsum = awork.tile([128, 1], F32, tag="rsum")
                    nc.vector.reciprocal(rsum, ssum)
                    asb_r = awork.tile([128, T], F32R, tag="asb_r")
                    nc.vector.tensor_scalar_mul(asb_r, asb, rsum)
                    for kt in range(2):
                        tp2 = aps_tr.tile([128, 128], F32R, tag="tr")
                        nc.tensor.transpose(tp2, asb_r[:, kt * 128:(kt + 1) * 128],
                                            identr)
                        nc.vector.tensor_copy(
                            attn_t[:, kt, qt * 128:(qt + 1) * 128], tp2)
                per_head.append((vht, attn_t))
            # PV for the pair, token-major: out[q, d] per qt into one psum tile
            for qt in range(2):
                pvp = aps_pv.tile([128, 2 * HD], F32, tag="pv")
                for j, (vht, attn_t) in enumerate(per_head):
                    for kt in range(2):
                        nc.tensor.matmul(pvp[:, j * HD:(j + 1) * HD],
                                         attn_t[:, kt, qt * 128:(qt + 1) * 128],
                                         vht[:, kt, :],
                                         start=(kt == 0), stop=(kt == 1))
                nc.scalar.copy(ao_tm[:, qt, pair * 2 * HD:(pair + 1) * 2 * HD], pvp)
        # transpose ao back to feature-major for the output projection
        ao = aw1.tile([128, IC, T], F32R, tag="ao")
        for oc in range(IC):
            for qt in range(2):
                tpo = aps_tr.tile([128, 128], F32R, tag="tr")
                nc.tensor.transpose(tpo, ao_tm[:, qt, oc * 128:(oc + 1) * 128],
                                    identr)
                nc.vector.tensor_copy(ao[:, oc, qt * 128:(qt + 1) * 128], tpo)
        # out-proj + bias + residual
        for ot in range(IC):
            wt = wpool.tile([128, IC, 128], F32R, tag="wsl")
            nc.sync.dma_start(out=wt, in_=w_out[ot])
            ps = aps_mm.tile([128, T], F32, tag="mm")
            for ic in range(IC):
                nc.tensor.matmul(ps, wt[:, ic, :], ao[:, ic, :],
                                 start=(ic == 0), stop=(ic == IC - 1))
            nc.vector.scalar_tensor_tensor(new_resid[:, ot, :], in0=ps,
                                           scalar=b_out[:, ot:ot + 1],
                                           in1=old_resid[:, ot, :],
                                           op0=ALU.add, op1=ALU.add)

    def gate(s, tokens_fm, w1sb, w2sb, bsb, ctx):
        """ptm[:, 2s:2s+2, :] = softmax_E(tokens.W1 + ctx.W2 + b); also pfm."""
        ct_ps = aps_sm.tile([1, E], F32, tag="sm")
        for ic in range(IC):
            nc.tensor.matmul(ct_ps, ctx[:, ic, :], w2sb[:, ic, :],
                             start=(ic == 0), stop=(ic == IC - 1))
        crow = awork.tile([1, E], F32, tag="crow")
        nc.vector.tensor_add(crow, ct_ps, bsb)
        crow_bc = awork.tile([128, E], F32, tag="crow_bc")
        nc.gpsimd.partition_broadcast(crow_bc, crow)
        for tt in range(2):
            lg_ps = aps_tr.tile([128, E], F32, tag="tr")
            for ic in range(IC):
                nc.tensor.matmul(lg_ps, tokens_fm[:, ic, tt * 128:(tt + 1) * 128],
                                 w1sb[:, ic, :],
                                 start=(ic == 0), stop=(ic == IC - 1))
            lg = awork.tile([128, E], F32, tag="lg")
            nc.vector.tensor_add(lg, lg_ps, crow_bc)
            nm = awork.tile([128, 1], F32, tag="gnm")
            nc.vector.reduce_max(nm, lg, axis=AX.X, negate=True)
            gs = awork.tile([128, 1], F32, tag="gs")
            nc.scalar.activation(ptm[:, 2 * s + tt, :], lg, AF.Exp, bias=nm,
                                 accum_out=gs)
            gr = awork.tile([128, 1], F32, tag="gr")
            nc.vector.reciprocal(gr, gs)
            nc.vector.tensor_scalar_mul(ptm[:, 2 * s + tt, :],
                                        ptm[:, 2 * s + tt, :], gr)
            tp = aps_tr.tile([E, 128], F32, tag="tr")
            nc.tensor.transpose(tp, ptm[:, 2 * s + tt, :], ident)
            nc.vector.tensor_copy(pfm[:, 2 * s + tt, :], tp)

    # ---- phase-1 body ---------------------------------------------------
    xq0 = aw1.tile([128, IC, T], F32R, tag="xq0")
    nc.sync.dma_start(out=xq0, in_=dt["xq"])
    xt0 = aw1.tile([128, IC, T], F32R, tag="xt0")
    nc.sync.dma_start(out=xt0, in_=dt["xt"])

    qn = aw1.tile([128, IC, T], F32R, tag="qn")
    ln_fm(qn, xq0, lnp["g_lnq"], lnp["b_lnq"])
    q1 = aw1.tile([128, IC, T], F32R, tag="q1")
    attention(q1, xq0, qn, qn, dt["w_sa"], b_sa.rearrange("p a b -> p (a b)"),
              dt["w_sao"], b_sao)

    qn2 = aw1.tile([128, IC, T], F32R, tag="qn2")
    ln_fm(qn2, q1, lnp["g_lnc"], lnp["b_lnc"])
    attention(q2, q1, qn2, xi0, dt["w_ca"], b_ca.rearrange("p a b -> p (a b)"),
              dt["w_cao"], b_cao)

    # contexts: mean over tokens
    ictx = awork.tile([128, IC, 1], F32R, tag="ictx")
    tctx = awork.tile([128, IC, 1], F32R, tag="tctx")
    with nc.allow_low_precision(reason="f32r shares f32 bits; DVE sum is fp32"):
        for ic in range(IC):
            nc.vector.reduce_sum(ictx[:, ic, :], xi0[:, ic, :], axis=AX.X)
            nc.vector.reduce_sum(tctx[:, ic, :], xt0[:, ic, :], axis=AX.X)
    nc.scalar.mul(ictx.rearrange("p a b -> p (a b)"),
                  ictx.rearrange("p a b -> p (a b)"), 1.0 / T)
    nc.scalar.mul(tctx.rearrange("p a b -> p (a b)"),
                  tctx.rearrange("p a b -> p (a b)"), 1.0 / T)

    # routers: query stream uses txt gate on q2; image stream uses img gate
    gate(0, q2, w_tg1, w_tg2, b_tg, ictx)
    gate(1, xi0, w_ig1, w_ig2, b_ig, tctx)
    # probs / 256 compensates the x256 fp8 weight scale of w2
    nc.scalar.mul(ps8.rearrange("p a b -> p (a b)"),
                  ptm.rearrange("p a b -> p (a b)"), 1.0 / WS)

    # moe input (fp8): [ lnf(q2) ; xi0 ]
    lnf_scr = aw1.tile([128, IC, T], F32, tag="lnf_scr")
    ln_fm(lnf_scr, q2, lnp["g_lnf"], lnp["b_lnf"],
          dst8=x8.rearrange("p (s c) t -> p s c t", s=2)[:, 0])
    nc.vector.tensor_copy(x8[:, :, T:T2].rearrange("p c t -> p (c t)"),
                          xi0.rearrange("p c t -> p (c t)"))

    # moe accumulator (token-major) init: residual + sum_e probs_e * b2_e
    b2mr = awork.tile([E, H], F32R, tag="b2mr")
    nc.vector.tensor_copy(b2mr, b2m)
    pfmr = awork.tile([E, NTT, 128], F32R, tag="pfmr")
    nc.vector.tensor_copy(pfmr, pfm)
    for tt in range(NTT):
        src = q2 if tt < 2 else xi0
        t0 = (tt % 2) * 128
        b2ps = [aps_pv.tile([128, 512], F32, tag="pv", name=f"b2ps_{tt}_{oh}")
                for oh in range(2)]
        for oh in range(2):
            nc.tensor.matmul(b2ps[oh], pfmr[:, tt, :],
                             b2mr[:, oh * 512:(oh + 1) * 512],
                             start=True, stop=True)
        for oc in range(IC):
            tp = aps_tr.tile([128, 128], F32R, tag="tr")
            nc.tensor.transpose(tp, src[:, oc, t0:t0 + 128], identr)
            nc.vector.tensor_add(
                acc[:, tt, oc * 128:(oc + 1) * 128], tp.bitcast(F32),
                b2ps[oc // 4][:, (oc % 4) * 128:(oc % 4 + 1) * 128])

    for p in (wpool, aw1, awork, aps_sm, aps_pv, aps_tr, aps_mm):
        p.release()

    # ====================================================================
    # phase 2: dense fp8 DoubleRow MoE over both streams (512 tokens)
    # ====================================================================
    mps_h = tc.alloc_tile_pool(name="mps_h", bufs=2, space="PSUM")
    mps_o = tc.alloc_tile_pool(name="mps_o", bufs=4, space="PSUM")
    hpool = tc.alloc_tile_pool(name="hpool", bufs=2)
    mw1 = tc.alloc_tile_pool(name="mw1", bufs=5)
    mw2 = tc.alloc_tile_pool(name="mw2", bufs=18)

    for e in range(E):
        # GEMM1: h = gelu(x @ W1 / 256 + b1), f-major fp8 [128, FT, T2]
        w1t = []
        for k in range(KT1):
            t = mw1.tile([128, 2, F], FP8, tag="w1sl")
            nc.sync.dma_start(out=t, in_=dt["w1"][e, k])
            w1t.append(t)
        h8 = hpool.tile([128, FT, T2], FP8, tag="h8")
        for ft in range(FT):
            hps = mps_h.tile([128, T2], F32, tag="h")
            for k in range(KT1):
                nc.tensor.matmul(hps, w1t[k][:, :, ft * 128:(ft + 1) * 128],
                                 x8[:, 2 * k:2 * k + 2, :],
                                 start=(k == 0), stop=(k == KT1 - 1),
                                 perf_mode=DR)
            nc.scalar.activation(h8[:, ft, :], hps, AF.Gelu,
                                 bias=b1f[:, e, ft:ft + 1], scale=1.0 / WS)
        # GEMM2: o_tm = h.T @ W2 (h stationary -> token-major out);
        # acc += probs/256 * o
        w2t = []
        for k in range(KT2):
            t = mw2.tile([128, 2, H], FP8, tag="w2sl")
            nc.sync.dma_start(out=t, in_=dt["w2"][e, k])
            w2t.append(t)
        for tt in range(NTT):
            ops_ = [mps_o.tile([128, 512], F32, tag="o", name=f"o_{e}_{tt}_{oh}")
                    for oh in range(2)]
            for k in range(KT2):
                hslice = h8[:, 2 * k:2 * k + 2, tt * 128:(tt + 1) * 128]
                for oh in range(2):
                    nc.tensor.matmul(ops_[oh], hslice,
                                     w2t[k][:, :, oh * 512:(oh + 1) * 512],
                                     start=(k == 0), stop=(k == KT2 - 1),
                                     perf_mode=DR)
            for oh in range(2):
                nc.vector.scalar_tensor_tensor(
                    acc[:, tt, oh * 512:(oh + 1) * 512], in0=ops_[oh],
                    scalar=ps8[:, tt, e:e + 1],
                    in1=acc[:, tt, oh * 512:(oh + 1) * 512],
                    op0=ALU.mult, op1=ALU.add)

    # ---- outputs: already token-major ----------------------------------
    nc.sync.dma_start(out=dt["oq"], in_=acc[:, 0:2, :])
    nc.sync.dma_start(out=dt["oi"], in_=acc[:, 2:4, :])

    for p in (mw2, mw1, hpool, mps_o, mps_h, persist):
        p.release()


# ----------------------------------------------------------------------------
# host-side prep + run
# ----------------------------------------------------------------------------

_NC = None
LAST_EXEC_NS = None


def _get_nc():
    global _NC
    if _NC is None:
        _NC = _build_program()
    return _NC


def _prep_inputs(inp):
    """Build the per-core in_maps from the full (unsharded) numpy inputs."""
    f = np.float32

    def c(a):
        return np.ascontiguousarray(a, dtype=f)

    def q8(a):  # scale + quantize to TRN e4m3
        return np.ascontiguousarray(
            (np.asarray(a, np.float32) * WS).astype(E4NP))

    shared = {}
    shared["w_sa"] = c(inp["sa_in_w"].reshape(3, 8, 128, IC, 128).transpose(0, 1, 4, 3, 2))
    shared["b_sa"] = c(inp["sa_in_b"].reshape(3, IC, 128).transpose(2, 0, 1))
    shared["w_sao"] = c(inp["sa_out_w"].reshape(8, 128, IC, 128).transpose(0, 3, 2, 1))
    shared["b_sao"] = c(inp["sa_out_b"].reshape(IC, 128).T)
    shared["w_ca"] = c(inp["ca_in_w"].reshape(3, 8, 128, IC, 128).transpose(0, 1, 4, 3, 2))
    shared["b_ca"] = c(inp["ca_in_b"].reshape(3, IC, 128).transpose(2, 0, 1))
    shared["w_cao"] = c(inp["ca_out_w"].reshape(8, 128, IC, 128).transpose(0, 3, 2, 1))
    shared["b_cao"] = c(inp["ca_out_b"].reshape(IC, 128).T)
    shared["w_ig1"] = c(inp["img_gate_w"][:, :H].T.reshape(IC, 128, E).transpose(1, 0, 2))
    shared["w_ig2"] = c(inp["img_gate_w"][:, H:].T.reshape(IC, 128, E).transpose(1, 0, 2))
    shared["b_ig"] = c(inp["img_gate_b"][None, :])
    shared["w_tg1"] = c(inp["txt_gate_w"][:, :H].T.reshape(IC, 128, E).transpose(1, 0, 2))
    shared["w_tg2"] = c(inp["txt_gate_w"][:, H:].T.reshape(IC, 128, E).transpose(1, 0, 2))
    shared["b_tg"] = c(inp["txt_gate_b"][None, :])
    for n, k in (("g_lnq", "lnq_g"), ("b_lnq", "lnq_b"), ("g_lnc", "lnc_g"),
                 ("b_lnc", "lnc_b"), ("g_lnf", "lnf_g"), ("b_lnf", "lnf_b")):
        shared[n] = c(inp[k].reshape(IC, 128).T)
    # fp8 DoubleRow pair layout: [e, k, part(128), pair, out]
    shared["w1"] = q8(inp["ew1"].reshape(E, KT1, 2, 128, F).transpose(0, 1, 3, 2, 4))
    shared["b1"] = c(inp["eb1"].reshape(E, FT, 128).transpose(2, 0, 1))
    shared["w2"] = q8(inp["ew2"].reshape(E, KT2, 2, 128, H).transpose(0, 1, 3, 2, 4))
    shared["b2"] = c(inp["eb2"])

    def fm(a):  # [T, H] -> [128, IC, T]
        return c(a.T.reshape(IC, 128, T).transpose(1, 0, 2))

    in_maps = []
    for b in range(B):
        m = dict(shared)
        m["xq"] = fm(np.asarray(inp["query_tokens"][b]))
        m["xi"] = fm(np.asarray(inp["image_tokens"][b]))
        m["xt"] = fm(np.asarray(inp["text_context"][b]))
        in_maps.append(m)
    return in_maps


def _run(inp, trace=False):
    global LAST_EXEC_NS
    nc = _get_nc()
    in_maps = _prep_inputs(inp)
    res = run_bass_kernel_spmd(nc, in_maps, core_ids=list(range(B)), trace=trace)
    LAST_EXEC_NS = res.exec_time_ns
    oq = np.empty((B, T, H), np.float32)
    oi = np.empty((B, T, H), np.float32)
    for b in range(B):
        oq[b] = res.results[b]["oq"].transpose(1, 0, 2).reshape(T, H)
        oi[b] = res.results[b]["oi"].transpose(1, 0, 2).reshape(T, H)
    return oq, oi


def kernel(**inputs):
    return _run(inputs, trace=False)


# revision 7
# speedup vs baseline: 1.6668x; 1.6668x over previous
"""CrossModalMoELayer Trainium2 Bass kernel.

Sharding: data-parallel over batch B=8 across the 8 NeuronCores (one batch
element per core). Each core runs the full layer for its batch element:
self-attention, cross-attention, gating, and the dense 8-expert MoE.

Attention/gating run in float32r (full-rate fp32 PE mode). The MoE - the
dominant compute - runs in fp8 (e4m3) with DoubleRow perf mode (2 fp8
MACs/cell/cycle, ~2x PE throughput). Expert weights are host-quantized to
e4m3 with a x256 scale (keeps the 0.02-scale weights out of the subnormal
range); the scale is folded back out in the gelu (scale=1/256) and the
router-prob accumulate (probs/256).

Layouts on device:
  feature-major ("fm"): [feat_part=128, feat_chunk, tokens]  - activations
  MoE output accumulates token-major: [tok_part=128, tok_tile, feature] so
  router probs apply as native per-partition scalars and the final store
  needs no transposes.

kernel(**inputs) takes the FULL unsharded inputs (numpy, keyed as in
setup_inputs()) and returns the full (query_tokens, image_tokens) tuple.
"""

import numpy as np
import ml_dtypes

import concourse.bass as bass
import concourse.tile as tile
from concourse import bacc, mybir
from concourse.bass_utils import run_bass_kernel_spmd
from concourse.masks import make_identity

B, T, H, NH, HD, F, E = 8, 256, 1024, 16, 64, 4096, 8
IC = H // 128          # 8 feature chunks of the model dim
FT = F // 128          # 32 feature chunks of the FFN dim
KT1 = IC // 2          # 4 DoubleRow k-tiles for GEMM1 (contraction H)
KT2 = FT // 2          # 16 DoubleRow k-tiles for GEMM2 (contraction F)
NTT = 4                # token tiles of 128 across both streams
T2 = 2 * T             # query tokens + image tokens concatenated
EPS = 1e-5
WS = 256.0             # fp8 weight scale

F32 = mybir.dt.float32
F32R = mybir.dt.float32r
FP8 = mybir.dt.float8e4
E4NP = ml_dtypes.float8_e4m3
AX = mybir.AxisListType
ALU = mybir.AluOpType
AF = mybir.ActivationFunctionType
DR = mybir.MatmulPerfMode.DoubleRow


# ----------------------------------------------------------------------------
# program builder
# ----------------------------------------------------------------------------

def _build_program():
    nc = bacc.Bacc(
        "TRN2",
        target_bir_lowering=False,
        debug=False,
        enable_asserts=False,
        num_devices=8,
    )

    dt = {}

    def din(name, shape, d=F32):
        dt[name] = nc.dram_tensor(name, list(shape), d, kind="ExternalInput").ap()
        return dt[name]

    def dout(name, shape):
        dt[name] = nc.dram_tensor(name, list(shape), F32, kind="ExternalOutput").ap()
        return dt[name]

    # activations (per core)
    din("xq", (128, IC, T), F32R)
    din("xi", (128, IC, T), F32R)
    din("xt", (128, IC, T), F32R)
    # attention weights: [proj, ot, i(128), ic, o(128)]
    din("w_sa", (3, 8, 128, IC, 128), F32R)
    din("b_sa", (128, 3, IC))
    din("w_sao", (8, 128, IC, 128), F32R)
    din("b_sao", (128, IC))
    din("w_ca", (3, 8, 128, IC, 128), F32R)
    din("b_ca", (128, 3, IC))
    din("w_cao", (8, 128, IC, 128), F32R)
    din("b_cao", (128, IC))
    # gates
    din("w_ig1", (128, IC, E), F32R)
    din("w_ig2", (128, IC, E), F32R)
    din("b_ig", (1, E))
    din("w_tg1", (128, IC, E), F32R)
    din("w_tg2", (128, IC, E), F32R)
    din("b_tg", (1, E))
    # layernorms [128, IC]
    for n in ("g_lnq", "b_lnq", "g_lnc", "b_lnc", "g_lnf", "b_lnf"):
        din(n, (128, IC))
    # experts (fp8, x256-scaled, DoubleRow pair layout)
    din("w1", (E, KT1, 128, 2, F), FP8)        # [e, k, i(128), pair, f]
    din("b1", (128, E, FT))
    din("w2", (E, KT2, 128, 2, H), FP8)        # [e, k, f(128), pair, o]
    din("b2", (E, H))
    # outputs token-major [t(128), tt, o]
    dout("oq", (128, 2, H))
    dout("oi", (128, 2, H))

    with tile.TileContext(nc) as tc:
        _trace_kernel(nc, tc, dt)

    nc.compile()
    return nc


def _trace_kernel(nc, tc, dt):
    persist = tc.alloc_tile_pool(name="persist", bufs=1)

    # ---- constants + small params --------------------------------------
    ident = persist.tile([128, 128], F32, tag="ident")
    make_identity(nc, ident)
    ones_f = persist.tile([128, 1], F32, tag="ones_f")
    nc.vector.memset(ones_f, 1.0)
    ones = persist.tile([128, 1], F32R, tag="ones")
    nc.vector.tensor_copy(ones, ones_f)
    identr = persist.tile([128, 128], F32R, tag="identr")
    nc.vector.tensor_copy(identr, ident)
    eps_t = persist.tile([1, 1], F32, tag="eps")
    nc.vector.memset(eps_t, EPS)

    def load(name, shape, d=F32, pool=persist):
        t = pool.tile(list(shape), d, tag=f"ld_{name}")
        nc.sync.dma_start(out=t, in_=dt[name])
        return t

    xi0 = load("xi", (128, IC, T), F32R)
    b_sa = load("b_sa", (128, 3, IC))
    b_sao = load("b_sao", (128, IC))
    b_ca = load("b_ca", (128, 3, IC))
    b_cao = load("b_cao", (128, IC))
    w_ig1 = load("w_ig1", (128, IC, E), F32R)
    w_ig2 = load("w_ig2", (128, IC, E), F32R)
    b_ig = load("b_ig", (1, E))
    w_tg1 = load("w_tg1", (128, IC, E), F32R)
    w_tg2 = load("w_tg2", (128, IC, E), F32R)
    b_tg = load("b_tg", (1, E))
    lnp = {n: load(n, (128, IC)) for n in
           ("g_lnq", "b_lnq", "g_lnc", "b_lnc", "g_lnf", "b_lnf")}
    b1f = load("b1", (128, E, FT))
    b2m = load("b2", (E, H))

    # persistent activations
    q2 = persist.tile([128, IC, T], F32R, tag="q2")          # query after CA
    x8 = persist.tile([128, IC, T2], FP8, tag="x8")          # fp8 [lnf(q2); xi0]
    ptm = persist.tile([128, NTT, E], F32, tag="ptm")        # router probs (tm)
    ps8 = persist.tile([128, NTT, E], F32, tag="ps8")        # probs / 256
    pfm = persist.tile([E, NTT, 128], F32, tag="pfm")        # probs (fm, for b2)
    acc = persist.tile([128, NTT, H], F32, tag="acc")        # MoE accum (tm)

    # ====================================================================
    # phase 1: attention + gating + lnf (own pools, released before MoE)
    # ====================================================================
    aps_mm = tc.alloc_tile_pool(name="aps_mm", bufs=3, space="PSUM")
    aps_tr = tc.alloc_tile_pool(name="aps_tr", bufs=2, space="PSUM")
    aps_pv = tc.alloc_tile_pool(name="aps_pv", bufs=2, space="PSUM")
    aps_sm = tc.alloc_tile_pool(name="aps_sm", bufs=1, space="PSUM")
    awork = tc.alloc_tile_pool(name="awork", bufs=2)
    aw1 = tc.alloc_tile_pool(name="aw1", bufs=1)
    wpool = tc.alloc_tile_pool(name="wpool", bufs=4)

    def ln_fm(dst, src, g, b, dst8=None):
        """dst[:, ic, :] = LN over features of src (fm layout [128, IC, T]).

        If dst8 is given, the normalized result is additionally written to
        dst8 (fp8) with the fp32 intermediate kept in dst.
        """
        ntok = src.shape[2]
        sum_ps = aps_sm.tile([1, ntok], F32, tag="sm")
        for ic in range(IC):
            nc.tensor.matmul(sum_ps, ones, src[:, ic, :],
                             start=(ic == 0), stop=(ic == IC - 1))
        mean = awork.tile([1, ntok], F32, tag="ln_mean")
        nc.scalar.mul(mean, sum_ps, 1.0 / H)
        sumsq_ps = aps_sm.tile([1, ntok], F32, tag="sm")
        for ic in range(IC):
            xsq = awork.tile([128, ntok], F32R, tag="ln_xsq")
            nc.scalar.activation(xsq, src[:, ic, :], AF.Square)
            nc.tensor.matmul(sumsq_ps, ones, xsq,
                             start=(ic == 0), stop=(ic == IC - 1))
        msq = awork.tile([1, ntok], F32, tag="ln_msq")
        nc.vector.tensor_mul(msq, mean, mean)
        var = awork.tile([1, ntok], F32, tag="ln_var")
        nc.vector.scalar_tensor_tensor(var, in0=sumsq_ps, scalar=1.0 / H,
                                       in1=msq, op0=ALU.mult, op1=ALU.subtract)
        std = awork.tile([1, ntok], F32, tag="ln_std")
        nc.scalar.activation(std, var, AF.Sqrt, bias=eps_t)
        rstd = awork.tile([1, ntok], F32, tag="ln_rstd")
        nc.vector.reciprocal(rstd, std)
        negc = awork.tile([1, ntok], F32, tag="ln_negc")
        nc.vector.scalar_tensor_tensor(negc, in0=mean, scalar=-1.0,
                                       in1=rstd, op0=ALU.mult, op1=ALU.mult)
        a_bc = awork.tile([128, ntok], F32, tag="ln_abc")
        nc.gpsimd.partition_broadcast(a_bc, rstd)
        c_bc = awork.tile([128, ntok], F32, tag="ln_cbc")
        nc.gpsimd.partition_broadcast(c_bc, negc)
        for ic in range(IC):
            nc.vector.tensor_mul(dst[:, ic, :], src[:, ic, :], a_bc)
            nc.vector.tensor_add(dst[:, ic, :], dst[:, ic, :], c_bc)
            out_ic = dst[:, ic, :] if dst8 is None else dst8[:, ic, :]
            nc.vector.tensor_scalar(out=out_ic, in0=dst[:, ic, :],
                                    scalar1=g[:, ic:ic + 1], scalar2=b[:, ic:ic + 1],
                                    op0=ALU.mult, op1=ALU.add)

    def proj_fm(dst, src, w_dram_ot, bias, bias_col):
        """dst[:, ot, :] = W @ src + b  (fm in, fm out)."""
        ntok = src.shape[2]
        for ot in range(IC):
            wt = wpool.tile([128, IC, 128], F32R, tag="wsl")
            nc.sync.dma_start(out=wt, in_=w_dram_ot(ot))
            ps = aps_mm.tile([128, ntok], F32, tag="mm")
            for ic in range(IC):
                nc.tensor.matmul(ps, wt[:, ic, :], src[:, ic, :],
                                 start=(ic == 0), stop=(ic == IC - 1))
            nc.scalar.add(dst[:, ot, :], ps, bias[:, bias_col(ot)])

    def attention(new_resid, old_resid, qsrc, kvsrc, w_in, b_in, w_out, b_out):
        """new_resid = old_resid + out_proj(MHA(q=qsrc, kv=kvsrc)); all fm."""
        qf = aw1.tile([128, IC, T], F32R, tag="qf")
        kf = aw1.tile([128, IC, T], F32R, tag="kf")
        vf = aw1.tile([128, IC, T], F32R, tag="vf")
        proj_fm(qf, qsrc, lambda ot: w_in[0, ot], b_in, lambda ot: slice(0 * IC + ot, 0 * IC + ot + 1))
        proj_fm(kf, kvsrc, lambda ot: w_in[1, ot], b_in, lambda ot: slice(1 * IC + ot, 1 * IC + ot + 1))
        proj_fm(vf, kvsrc, lambda ot: w_in[2, ot], b_in, lambda ot: slice(2 * IC + ot, 2 * IC + ot + 1))
        # attention output, token-major: ao_tm[t(128), qt, h*64+d]
        ao_tm = aw1.tile([128, 2, H], F32R, tag="ao_tm")
        for pair in range(NH // 2):
            per_head = []
            for h in (2 * pair, 2 * pair + 1):
                base = (h % 2) * HD
                c = h // 2
                qh = qf[base:base + HD, c, :]
                kh = kf[base:base + HD, c, :]
                vh = vf[base:base + HD, c, :]
                idn = identr[base:base + HD, base:base + HD]
                # vh^T : [T, HD] in two 128-token tiles
                vht = awork.tile([128, 2, HD], F32R, tag="vht",
                                 name=f"vht_{h}")
                for kt in range(2):
                    tp = aps_tr.tile([128, HD], F32R, tag="tr")
                    nc.tensor.transpose(tp, vh[:, kt * 128:(kt + 1) * 128], idn)
                    nc.vector.tensor_copy(vht[:, kt, :], tp)
                attn_t = awork.tile([128, 2, T], F32R, tag="attnT",
                                    name=f"attnT_{h}")
                for qt in range(2):
                    sc = aps_mm.tile([128, T], F32, tag="mm")
                    nc.tensor.matmul(sc, qh[:, qt * 128:(qt + 1) * 128], kh,
                                     start=True, stop=True)
                    nmax = awork.tile([128, 1], F32, tag="nmax")
                    nc.vector.reduce_max(nmax, sc, axis=AX.X, negate=True)
                    nmax2 = awork.tile([128, 1], F32, tag="nmax2")
                    nc.scalar.mul(nmax2, nmax, 0.125)
                    asb = awork.tile([128, T], F32, tag="asb")
                    ssum = awork.tile([128, 1], F32, tag="ssum")
                    nc.scalar.activation(asb, sc, AF.Exp, bias=nmax2, scale=0.125,
                                         accum_out=ssum)
                    rsum = awork.tile([128, 1], F32, tag="rsum")
                    nc.vector.reciprocal(rsum, ssum)
                    asb_r = awork.tile([128, T], F32R, tag="asb_r")
                    nc.vector.tensor_scalar_mul(asb_r, asb, rsum)
                    for kt in range(2):
                        tp2 = aps_tr.tile([128, 128], F32R, tag="tr")
                        nc.tensor.transpose(tp2, asb_r[:, kt * 128:(kt + 1) * 128],
                                            identr)
                        nc.vector.tensor_copy(
                            attn_t[:, kt, qt * 128:(qt + 1) * 128], tp2)
                per_head.append((vht, attn_t))
            # PV for the pair, token-major: out[q, d] per qt into one psum tile
            for qt in range(2):
                pvp = aps_pv.tile([128, 2 * HD], F32, tag="pv")
                for j, (vht, attn_t) in enumerate(per_head):
                    for kt in range(2):
                        nc.tensor.matmul(pvp[:, j * HD:(j + 1) * HD],
                                         attn_t[:, kt, qt * 128:(qt + 1) * 128],
                                         vht[:, kt, :],
                                         start=(kt == 0), stop=(kt == 1))
                nc.scalar.copy(ao_tm[:, qt, pair * 2 * HD:(pair + 1) * 2 * HD], pvp)
        # transpose ao back to feature-major for the output projection
        ao = aw1.tile([128, IC, T], F32R, tag="ao")
        for oc in range(IC):
            for qt in range(2):
                tpo = aps_tr.tile([128, 128], F32R, tag="tr")
                nc.tensor.transpose(tpo, ao_tm[:, qt, oc * 128:(oc + 1) * 128],
                                    identr)
                nc.vector.tensor_copy(ao[:, oc, qt * 128:(qt + 1) * 128], tpo)
        # out-proj + bias + residual
        for ot in range(IC):
            wt = wpool.tile([128, IC, 128], F32R, tag="wsl")
            nc.sync.dma_start(out=wt, in_=w_out[ot])
            ps = aps_mm.tile([128, T], F32, tag="mm")
            for ic in range(IC):
                nc.tensor.matmul(ps, wt[:, ic, :], ao[:, ic, :],
                                 start=(ic == 0), stop=(ic == IC - 1))
            nc.vector.scalar_tensor_tensor(new_resid[:, ot, :], in0=ps,
                                           scalar=b_out[:, ot:ot + 1],
                                           in1=old_resid[:, ot, :],
                                           op0=ALU.add, op1=ALU.add)

    def gate(s, tokens_fm, w1sb, w2sb, bsb, ctx):
        """ptm[:, 2s:2s+2, :] = softmax_E(tokens.W1 + ctx.W2 + b); also pfm."""
        ct_ps = aps_sm.tile([1, E], F32, tag="sm")
        for ic in range(IC):
            nc.tensor.matmul(ct_ps, ctx[:, ic, :], w2sb[:, ic, :],
                             start=(ic == 0), stop=(ic == IC - 1))
        crow = awork.tile([1, E], F32, tag="crow")
        nc.vector.tensor_add(crow, ct_ps, bsb)
        crow_bc = awork.tile([128, E], F32, tag="crow_bc")
        nc.gpsimd.partition_broadcast(crow_bc, crow)
        for tt in range(2):
            lg_ps = aps_tr.tile([128, E], F32, tag="tr")
            for ic in range(IC):
                nc.tensor.matmul(lg_ps, tokens_fm[:, ic, tt * 128:(tt + 1) * 128],
                                 w1sb[:, ic, :],
                                 start=(ic == 0), stop=(ic == IC - 1))
            lg = awork.tile([128, E], F32, tag="lg")
            nc.vector.tensor_add(lg, lg_ps, crow_bc)
            nm = awork.tile([128, 1], F32, tag="gnm")
            nc.vector.reduce_max(nm, lg, axis=AX.X, negate=True)
            gs = awork.tile([128, 1], F32, tag="gs")
            nc.scalar.activation(ptm[:, 2 * s + tt, :], lg, AF.Exp, bias=nm,
                                 accum_out=gs)
            gr = awork.tile([128, 1], F32, tag="gr")
            nc.vector.reciprocal(gr, gs)
            nc.vector.tensor_scalar_mul(ptm[:, 2 * s + tt, :],
                                        ptm[:, 2 * s + tt, :], gr)
            tp = aps_tr.tile([E, 128], F32, tag="tr")
            nc.tensor.transpose(tp, ptm[:, 2 * s + tt, :], ident)
            nc.vector.tensor_copy(pfm[:, 2 * s + tt, :], tp)

    # ---- phase-1 body ---------------------------------------------------
    xq0 = aw1.tile([128, IC, T], F32R, tag="xq0")
    nc.sync.dma_start(out=xq0, in_=dt["xq"])
    xt0 = aw1.tile([128, IC, T], F32R, tag="xt0")
    nc.sync.dma_start(out=xt0, in_=dt["xt"])

    qn = aw1.tile([128, IC, T], F32R, tag="qn")
    ln_fm(qn, xq0, lnp["g_lnq"], lnp["b_lnq"])
    q1 = aw1.tile([128, IC, T], F32R, tag="q1")
    attention(q1, xq0, qn, qn, dt["w_sa"], b_sa.rearrange("p a b -> p (a b)"),
              dt["w_sao"], b_sao)

    qn2 = aw1.tile([128, IC, T], F32R, tag="qn2")
    ln_fm(qn2, q1, lnp["g_lnc"], lnp["b_lnc"])
    attention(q2, q1, qn2, xi0, dt["w_ca"], b_ca.rearrange("p a b -> p (a b)"),
              dt["w_cao"], b_cao)

    # contexts: mean over tokens
    ictx = awork.tile([128, IC, 1], F32R, tag="ictx")
    tctx = awork.tile([128, IC, 1], F32R, tag="tctx")
    with nc.allow_low_precision(reason="f32r shares f32 bits; DVE sum is fp32"):
        for ic in range(IC):
            nc.vector.reduce_sum(ictx[:, ic, :], xi0[:, ic, :], axis=AX.X)
            nc.vector.reduce_sum(tctx[:, ic, :], xt0[:, ic, :], axis=AX.X)
    nc.scalar.mul(ictx.rearrange("p a b -> p (a b)"),
                  ictx.rearrange("p a b -> p (a b)"), 1.0 / T)
    nc.scalar.mul(tctx.rearrange("p a b -> p (a b)"),
                  tctx.rearrange("p a b -> p (a b)"), 1.0 / T)

    # routers: query stream uses txt gate on q2; image stream uses img gate
    gate(0, q2, w_tg1, w_tg2, b_tg, ictx)
    gate(1, xi0, w_ig1, w_ig2, b_ig, tctx)
    # probs / 256 compensates the x256 fp8 weight scale of w2
    nc.scalar.mul(ps8.rearrange("p a b -> p (a b)"),
                  ptm.rearrange("p a b -> p (a b)"), 1.0 / WS)

    # moe input (fp8): [ lnf(q2) ; xi0 ]
    lnf_scr = aw1.tile([128, IC, T], F32, tag="lnf_scr")
    ln_fm(lnf_scr, q2, lnp["g_lnf"], lnp["b_lnf"], dst8=x8[:, :, 0:T])
    with nc.allow_low_precision(reason="fp8 MoE inputs by design"):
        nc.vector.tensor_copy(x8[:, :, T:T2], xi0)

    # moe accumulator (token-major) init: residual + sum_e probs_e * b2_e
    b2mr = awork.tile([E, H], F32R, tag="b2mr")
    nc.vector.tensor_copy(b2mr, b2m)
    pfmr = awork.tile([E, NTT, 128], F32R, tag="pfmr")
    nc.vector.tensor_copy(pfmr, pfm)
    for tt in range(NTT):
        src = q2 if tt < 2 else xi0
        t0 = (tt % 2) * 128
        b2sb = awork.tile([128, H], F32, tag="b2sb")
        for oh in range(2):
            b2ps = aps_mm.tile([128, 512], F32, tag="mm", name=f"b2ps_{tt}_{oh}")
            nc.tensor.matmul(b2ps, pfmr[:, tt, :],
                             b2mr[:, oh * 512:(oh + 1) * 512],
                             start=True, stop=True)
            nc.scalar.copy(b2sb[:, oh * 512:(oh + 1) * 512], b2ps)
        for oc in range(IC):
            tp = aps_tr.tile([128, 128], F32R, tag="tr")
            nc.tensor.transpose(tp, src[:, oc, t0:t0 + 128], identr)
            nc.vector.tensor_add(
                acc[:, tt, oc * 128:(oc + 1) * 128], tp.bitcast(F32),
                b2sb[:, oc * 128:(oc + 1) * 128])

    for p in (wpool, aw1, awork, aps_sm, aps_pv, aps_tr, aps_mm):
        p.release()

    # ====================================================================
    # phase 2: dense fp8 DoubleRow MoE over both streams (512 tokens)
    # ====================================================================
    mps_h = tc.alloc_tile_pool(name="mps_h", bufs=2, space="PSUM")
    mps_o = tc.alloc_tile_pool(name="mps_o", bufs=4, space="PSUM")
    hpool = tc.alloc_tile_pool(name="hpool", bufs=2)
    mw1 = tc.alloc_tile_pool(name="mw1", bufs=5)
    mw2 = tc.alloc_tile_pool(name="mw2", bufs=18)

    for e in range(E):
        # GEMM1: h = gelu(x @ W1 / 256 + b1), f-major fp8 [128, FT, T2]
        w1t = []
        for k in range(KT1):
            t = mw1.tile([128, 2, F], FP8, tag="w1sl")
            nc.sync.dma_start(out=t, in_=dt["w1"][e, k])
            w1t.append(t)
        h8 = hpool.tile([128, FT, T2], FP8, tag="h8")
        for ft in range(FT):
            hps = mps_h.tile([128, T2], F32, tag="h")
            for k in range(KT1):
                nc.tensor.matmul(hps, w1t[k][:, :, ft * 128:(ft + 1) * 128],
                                 x8[:, 2 * k:2 * k + 2, :],
                                 start=(k == 0), stop=(k == KT1 - 1),
                                 perf_mode=DR)
            nc.scalar.activation(h8[:, ft, :], hps, AF.Gelu,
                                 bias=b1f[:, e, ft:ft + 1], scale=1.0 / WS)
        # GEMM2: o_tm = h.T @ W2 (h stationary -> token-major out);
        # acc += probs/256 * o
        w2t = []
        for k in range(KT2):
            t = mw2.tile([128, 2, H], FP8, tag="w2sl")
            nc.sync.dma_start(out=t, in_=dt["w2"][e, k])
            w2t.append(t)
        for tt in range(NTT):
            ops_ = [mps_o.tile([128, 512], F32, tag="o", name=f"o_{e}_{tt}_{oh}")
                    for oh in range(2)]
            for k in range(KT2):
                hslice = h8[:, 2 * k:2 * k + 2, tt * 128:(tt + 1) * 128]
                for oh in range(2):
                    nc.tensor.matmul(ops_[oh], hslice,
                                     w2t[k][:, :, oh * 512:(oh + 1) * 512],
                                     start=(k == 0), stop=(k == KT2 - 1),
                                     perf_mode=DR)
            for oh in range(2):
                nc.vector.scalar_tensor_tensor(
                    acc[:, tt, oh * 512:(oh + 1) * 512], in0=ops_[oh],
                    scalar=ps8[:, tt, e:e + 1],
                    in1=acc[:, tt, oh * 512:(oh + 1) * 512],
                    op0=ALU.mult, op1=ALU.add)

    # ---- outputs: already token-major ----------------------------------
    nc.sync.dma_start(out=dt["oq"], in_=acc[:, 0:2, :])
    nc.sync.dma_start(out=dt["oi"], in_=acc[:, 2:4, :])

    for p in (mw2, mw1, hpool, mps_o, mps_h, persist):
        p.release()


# ----------------------------------------------------------------------------
# host-side prep + run
# ----------------------------------------------------------------------------

_NC = None
LAST_EXEC_NS = None


def _get_nc():
    global _NC
    if _NC is None:
        _NC = _build_program()
    return _NC


def _prep_inputs(inp):
    """Build the per-core in_maps from the full (unsharded) numpy inputs."""
    f = np.float32

    def c(a):
        return np.ascontiguousarray(a, dtype=f)

    def q8(a):  # scale + quantize to TRN e4m3 (clip to TRN e4m3 max normal)
        return np.ascontiguousarray(
            np.clip(np.asarray(a, np.float32) * WS, -240, 240).astype(E4NP))

    shared = {}
    shared["w_sa"] = c(inp["sa_in_w"].reshape(3, 8, 128, IC, 128).transpose(0, 1, 4, 3, 2))
    shared["b_sa"] = c(inp["sa_in_b"].reshape(3, IC, 128).transpose(2, 0, 1))
    shared["w_sao"] = c(inp["sa_out_w"].reshape(8, 128, IC, 128).transpose(0, 3, 2, 1))
    shared["b_sao"] = c(inp["sa_out_b"].reshape(IC, 128).T)
    shared["w_ca"] = c(inp["ca_in_w"].reshape(3, 8, 128, IC, 128).transpose(0, 1, 4, 3, 2))
    shared["b_ca"] = c(inp["ca_in_b"].reshape(3, IC, 128).transpose(2, 0, 1))
    shared["w_cao"] = c(inp["ca_out_w"].reshape(8, 128, IC, 128).transpose(0, 3, 2, 1))
    shared["b_cao"] = c(inp["ca_out_b"].reshape(IC, 128).T)
    shared["w_ig1"] = c(inp["img_gate_w"][:, :H].T.reshape(IC, 128, E).transpose(1, 0, 2))
    shared["w_ig2"] = c(inp["img_gate_w"][:, H:].T.reshape(IC, 128, E).transpose(1, 0, 2))
    shared["b_ig"] = c(inp["img_gate_b"][None, :])
    shared["w_tg1"] = c(inp["txt_gate_w"][:, :H].T.reshape(IC, 128, E).transpose(1, 0, 2))
    shared["w_tg2"] = c(inp["txt_gate_w"][:, H:].T.reshape(IC, 128, E).transpose(1, 0, 2))
    shared["b_tg"] = c(inp["txt_gate_b"][None, :])
    for n, k in (("g_lnq", "lnq_g"), ("b_lnq", "lnq_b"), ("g_lnc", "lnc_g"),
                 ("b_lnc", "lnc_b"), ("g_lnf", "lnf_g"), ("b_lnf", "lnf_b")):
        shared[n] = c(inp[k].reshape(IC, 128).T)
    # fp8 DoubleRow pair layout: [e, k, part(128), pair, out]
    shared["w1"] = q8(inp["ew1"].reshape(E, KT1, 2, 128, F).transpose(0, 1, 3, 2, 4))
    shared["b1"] = c(inp["eb1"].reshape(E, FT, 128).transpose(2, 0, 1))
    shared["w2"] = q8(inp["ew2"].reshape(E, KT2, 2, 128, H).transpose(0, 1, 3, 2, 4))
    shared["b2"] = c(inp["eb2"])

    def fm(a):  # [T, H] -> [128, IC, T]
        return c(a.T.reshape(IC, 128, T).transpose(1, 0, 2))

    in_maps = []
    for b in range(B):
        m = dict(shared)
        m["xq"] = fm(np.asarray(inp["query_tokens"][b]))
        m["xi"] = fm(np.asarray(inp["image_tokens"][b]))
        m["xt"] = fm(np.asarray(inp["text_context"][b]))
        in_maps.append(m)
    return in_maps


def _run(inp, trace=False):
    global LAST_EXEC_NS
    nc = _get_nc()
    in_maps = _prep_inputs(inp)
    res = run_bass_kernel_spmd(nc, in_maps, core_ids=list(range(B)), trace=trace)
    LAST_EXEC_NS = res.exec_time_ns
    oq = np.empty((B, T, H), np.float32)
    oi = np.empty((B, T, H), np.float32)
    for b in range(B):
        oq[b] = res.results[b]["oq"].transpose(1, 0, 2).reshape(T, H)
        oi[b] = res.results[b]["oi"].transpose(1, 0, 2).reshape(T, H)
    return oq, oi


def kernel(**inputs):
    return _run(inputs, trace=False)


# revision 14
# speedup vs baseline: 1.8764x; 1.1257x over previous
"""CrossModalMoELayer Trainium2 Bass kernel.

Sharding: data-parallel over batch B=8 across the 8 NeuronCores (one batch
element per core). Each core runs the full layer for its batch element:
self-attention, cross-attention, gating, and the dense 8-expert MoE.

Attention/gating run in float32r (full-rate fp32 PE mode). The MoE - the
dominant compute - runs in fp8 (e4m3) with DoubleRow perf mode (2 fp8
MACs/cell/cycle, ~2x PE throughput). Expert weights are host-quantized to
e4m3 with a x256 scale (keeps the 0.02-scale weights out of the subnormal
range); the scale is folded back out in the gelu (scale=1/256) and the
router-prob accumulate (probs/256).

Layouts on device:
  feature-major ("fm"): [feat_part=128, feat_chunk, tokens]  - activations
  MoE output accumulates token-major: [tok_part=128, tok_tile, feature] so
  router probs apply as native per-partition scalars and the final store
  needs no transposes.

kernel(**inputs) takes the FULL unsharded inputs (numpy, keyed as in
setup_inputs()) and returns the full (query_tokens, image_tokens) tuple.
"""

import numpy as np
import ml_dtypes

import concourse.bass as bass
import concourse.tile as tile
from concourse import bacc, mybir
from concourse.bass_utils import run_bass_kernel_spmd
from concourse.masks import make_identity

B, T, H, NH, HD, F, E = 8, 256, 1024, 16, 64, 4096, 8
IC = H // 128          # 8 feature chunks of the model dim
FT = F // 128          # 32 feature chunks of the FFN dim
KT1 = IC // 2          # 4 DoubleRow k-tiles for GEMM1 (contraction H)
KT2 = FT // 2          # 16 DoubleRow k-tiles for GEMM2 (contraction F)
NTT = 4                # token tiles of 128 across both streams
T2 = 2 * T             # query tokens + image tokens concatenated
EPS = 1e-5
WS = 256.0             # fp8 weight scale

F32 = mybir.dt.float32
F32R = mybir.dt.float32r
BF16 = mybir.dt.bfloat16
FP8 = mybir.dt.float8e4
E4NP = ml_dtypes.float8_e4m3
BFNP = ml_dtypes.bfloat16
AX = mybir.AxisListType
ALU = mybir.AluOpType
AF = mybir.ActivationFunctionType
DR = mybir.MatmulPerfMode.DoubleRow


# ----------------------------------------------------------------------------
# program builder
# ----------------------------------------------------------------------------

def _build_program():
    nc = bacc.Bacc(
        "TRN2",
        target_bir_lowering=False,
        debug=False,
        enable_asserts=False,
        num_devices=8,
    )

    dt = {}

    def din(name, shape, d=F32):
        dt[name] = nc.dram_tensor(name, list(shape), d, kind="ExternalInput").ap()
        return dt[name]

    def dout(name, shape):
        dt[name] = nc.dram_tensor(name, list(shape), F32, kind="ExternalOutput").ap()
        return dt[name]

    # activations (per core)
    din("xq", (128, IC, T), F32R)
    din("xi", (128, IC, T), F32R)
    din("xt", (128, IC, T), F32R)
    # attention weights (bf16): q/k proj [proj, ot, i(128), ic, o(128)],
    # v proj token-major moving [i(128), ic, d], out-proj [ot, i, ic, o]
    din("w_saqk", (2, 8, 128, IC, 128), BF16)
    din("wv_sa", (128, IC, H), BF16)
    din("b_sa", (128, 3, IC))
    din("w_sao", (8, 128, IC, 128), BF16)
    din("b_sao", (128, IC))
    din("w_caqk", (2, 8, 128, IC, 128), BF16)
    din("wv_ca", (128, IC, H), BF16)
    din("b_ca", (128, 3, IC))
    din("w_cao", (8, 128, IC, 128), BF16)
    din("b_cao", (128, IC))
    # gates
    din("w_ig1", (128, IC, E), F32R)
    din("w_ig2", (128, IC, E), F32R)
    din("b_ig", (1, E))
    din("w_tg1", (128, IC, E), F32R)
    din("w_tg2", (128, IC, E), F32R)
    din("b_tg", (1, E))
    # layernorms [128, IC]
    for n in ("g_lnq", "b_lnq", "g_lnc", "b_lnc", "g_lnf", "b_lnf"):
        din(n, (128, IC))
    # experts (fp8, x256-scaled, DoubleRow pair layout)
    din("w1", (E, KT1, 128, 2, F), FP8)        # [e, k, i(128), pair, f]
    din("b1", (128, E, FT))
    din("w2", (E, KT2, 128, 2, H), FP8)        # [e, k, f(128), pair, o]
    din("b2", (E, H))
    # outputs token-major [t(128), tt, o]
    dout("oq", (128, 2, H))
    dout("oi", (128, 2, H))

    with tile.TileContext(nc) as tc:
        _trace_kernel(nc, tc, dt)

    nc.compile()
    return nc


def _trace_kernel(nc, tc, dt):
    persist = tc.alloc_tile_pool(name="persist", bufs=1)

    # ---- constants + small params --------------------------------------
    ident = persist.tile([128, 128], F32, tag="ident")
    make_identity(nc, ident)
    ones_f = persist.tile([128, 1], F32, tag="ones_f")
    nc.vector.memset(ones_f, 1.0)
    ones = persist.tile([128, 1], F32R, tag="ones")
    nc.vector.tensor_copy(ones, ones_f)
    ones_b = persist.tile([128, 1], BF16, tag="ones_b")
    nc.vector.tensor_copy(ones_b, ones_f)
    identr = persist.tile([128, 128], F32R, tag="identr")
    nc.vector.tensor_copy(identr, ident)
    eps_t = persist.tile([1, 1], F32, tag="eps")
    nc.vector.memset(eps_t, EPS)

    def load(name, shape, d=F32, pool=persist):
        t = pool.tile(list(shape), d, tag=f"ld_{name}")
        nc.sync.dma_start(out=t, in_=dt[name])
        return t

    xi0 = load("xi", (128, IC, T), F32R)
    b_sa = load("b_sa", (128, 3, IC))
    b_sao = load("b_sao", (128, IC))
    b_ca = load("b_ca", (128, 3, IC))
    b_cao = load("b_cao", (128, IC))
    w_ig1 = load("w_ig1", (128, IC, E), F32R)
    w_ig2 = load("w_ig2", (128, IC, E), F32R)
    b_ig = load("b_ig", (1, E))
    w_tg1 = load("w_tg1", (128, IC, E), F32R)
    w_tg2 = load("w_tg2", (128, IC, E), F32R)
    b_tg = load("b_tg", (1, E))
    lnp = {n: load(n, (128, IC)) for n in
           ("g_lnq", "b_lnq", "g_lnc", "b_lnc", "g_lnf", "b_lnf")}
    b1f = load("b1", (128, E, FT))
    b2m = load("b2", (E, H))

    # persistent activations
    q2 = persist.tile([128, IC, T], F32R, tag="q2")          # query after CA
    x8 = persist.tile([128, IC, T2], FP8, tag="x8")          # fp8 [lnf(q2); xi0]
    ptm = persist.tile([128, NTT, E], F32, tag="ptm")        # router probs (tm)
    ps8 = persist.tile([128, NTT, E], F32, tag="ps8")        # probs / 256
    pfm = persist.tile([E, NTT, 128], F32, tag="pfm")        # probs (fm, for b2)
    acc = persist.tile([128, NTT, H], F32, tag="acc")        # MoE accum (tm)

    # ====================================================================
    # phase 1: attention + gating + lnf (own pools, released before MoE)
    # ====================================================================
    aps_mm = tc.alloc_tile_pool(name="aps_mm", bufs=3, space="PSUM")
    aps_tr = tc.alloc_tile_pool(name="aps_tr", bufs=2, space="PSUM")
    aps_pv = tc.alloc_tile_pool(name="aps_pv", bufs=2, space="PSUM")
    aps_sm = tc.alloc_tile_pool(name="aps_sm", bufs=1, space="PSUM")
    awork = tc.alloc_tile_pool(name="awork", bufs=2)
    aw1 = tc.alloc_tile_pool(name="aw1", bufs=1)
    wpool = tc.alloc_tile_pool(name="wpool", bufs=4)

    def ln_fm(dst, src, g, b, dst8=None):
        """dst[:, ic, :] = LN over features of src (fm layout [128, IC, T]).

        If dst8 is given, the normalized result is additionally written to
        dst8 (fp8) with the fp32 intermediate kept in dst.
        """
        ntok = src.shape[2]
        sum_ps = aps_sm.tile([1, ntok], F32, tag="sm")
        for ic in range(IC):
            nc.tensor.matmul(sum_ps, ones, src[:, ic, :],
                             start=(ic == 0), stop=(ic == IC - 1))
        mean = awork.tile([1, ntok], F32, tag="ln_mean")
        nc.scalar.mul(mean, sum_ps, 1.0 / H)
        sumsq_ps = aps_sm.tile([1, ntok], F32, tag="sm")
        for ic in range(IC):
            xsq = awork.tile([128, ntok], F32R, tag="ln_xsq")
            nc.scalar.activation(xsq, src[:, ic, :], AF.Square)
            nc.tensor.matmul(sumsq_ps, ones, xsq,
                             start=(ic == 0), stop=(ic == IC - 1))
        msq = awork.tile([1, ntok], F32, tag="ln_msq")
        nc.vector.tensor_mul(msq, mean, mean)
        var = awork.tile([1, ntok], F32, tag="ln_var")
        nc.vector.scalar_tensor_tensor(var, in0=sumsq_ps, scalar=1.0 / H,
                                       in1=msq, op0=ALU.mult, op1=ALU.subtract)
        std = awork.tile([1, ntok], F32, tag="ln_std")
        nc.scalar.activation(std, var, AF.Sqrt, bias=eps_t)
        rstd = awork.tile([1, ntok], F32, tag="ln_rstd")
        nc.vector.reciprocal(rstd, std)
        negc = awork.tile([1, ntok], F32, tag="ln_negc")
        nc.vector.scalar_tensor_tensor(negc, in0=mean, scalar=-1.0,
                                       in1=rstd, op0=ALU.mult, op1=ALU.mult)
        a_bc = awork.tile([128, ntok], F32, tag="ln_abc")
        nc.gpsimd.partition_broadcast(a_bc, rstd)
        c_bc = awork.tile([128, ntok], F32, tag="ln_cbc")
        nc.gpsimd.partition_broadcast(c_bc, negc)
        for ic in range(IC):
            nc.vector.tensor_mul(dst[:, ic, :], src[:, ic, :], a_bc)
            nc.vector.tensor_add(dst[:, ic, :], dst[:, ic, :], c_bc)
            out_ic = dst[:, ic, :] if dst8 is None else dst8[:, ic, :]
            nc.vector.tensor_scalar(out=out_ic, in0=dst[:, ic, :],
                                    scalar1=g[:, ic:ic + 1], scalar2=b[:, ic:ic + 1],
                                    op0=ALU.mult, op1=ALU.add)

    def attention(new_resid, old_resid, qsrc, kvsrc, w_qk, wv, b_in, w_out,
                  b_out):
        """new_resid = old_resid + out_proj(MHA(q=qsrc, kv=kvsrc)).

        qsrc/kvsrc are fm bf16.  Scores are computed k-major (st[k, q]) so
        softmax needs no PE transposes: exp without max-subtraction (score
        range is ±4 here), per-q sums via ones-matmul, and PV emits the
        attention output directly feature-major with 1/sum folded in after.
        V is projected token-major (x stationary, Wv^T moving); the V bias
        is folded into b_out host-side (softmax weights sum to 1).
        """
        qf = aw1.tile([128, IC, T], BF16, tag="qf")
        kf = aw1.tile([128, IC, T], BF16, tag="kf")
        for proj, dst, src in ((0, qf, qsrc), (1, kf, kvsrc)):
            for ot in range(IC):
                wt = wpool.tile([128, IC, 128], BF16, tag="wsl")
                nc.sync.dma_start(out=wt, in_=w_qk[proj, ot])
                ps = aps_mm.tile([128, T], F32, tag="mm")
                for ic in range(IC):
                    nc.tensor.matmul(ps, wt[:, ic, :], src[:, ic, :],
                                     start=(ic == 0), stop=(ic == IC - 1))
                nc.scalar.add(dst[:, ot, :], ps,
                              b_in[:, proj * IC + ot:proj * IC + ot + 1])
        # V projection, token-major: vht[t, kt, d]
        wvt = wpool.tile([128, IC, H], BF16, tag="wvt", bufs=2)
        nc.sync.dma_start(out=wvt, in_=wv)
        vht = aw1.tile([128, 2, H], BF16, tag="vht")
        for kt in range(2):
            for dh in range(2):
                ps = aps_mm.tile([128, 512], F32, tag="mm")
                for ic in range(IC):
                    nc.tensor.matmul(ps, kvsrc[:, ic, kt * 128:(kt + 1) * 128],
                                     wvt[:, ic, dh * 512:(dh + 1) * 512],
                                     start=(ic == 0), stop=(ic == IC - 1))
                nc.scalar.copy(vht[:, kt, dh * 512:(dh + 1) * 512], ps)
        # attention per head-pair; output directly feature-major
        ao = aw1.tile([128, IC, T], BF16, tag="ao")
        for pair in range(NH // 2):
            pv_ps = aps_pv.tile([128, T], F32, tag="pv")
            rbcs = []
            for j in range(2):
                h = 2 * pair + j
                b0 = j * HD
                st_sb = awork.tile([128, 2, T], BF16, tag="st", name=f"st_{h}")
                sum_ps = aps_sm.tile([1, T], F32, tag="sm")
                for kt in range(2):
                    st_ps = aps_tr.tile([128, T], F32, tag="tr")
                    nc.tensor.matmul(st_ps,
                                     kf[b0:b0 + HD, pair, kt * 128:(kt + 1) * 128],
                                     qf[b0:b0 + HD, pair, :],
                                     start=True, stop=True)
                    nc.scalar.activation(st_sb[:, kt, :], st_ps, AF.Exp,
                                         scale=0.125)
                    nc.tensor.matmul(sum_ps, ones_b, st_sb[:, kt, :],
                                     start=(kt == 0), stop=(kt == 1))
                rs = awork.tile([1, T], F32, tag="rs")
                nc.vector.reciprocal(rs, sum_ps)
                rbc = awork.tile([128, T], F32, tag="rbc",
                                 name=f"rbc_{pair}_{j}")
                nc.gpsimd.partition_broadcast(rbc, rs)
                rbcs.append(rbc)
                for kt in range(2):
                    nc.tensor.matmul(pv_ps[b0:b0 + HD, :],
                                     vht[:, kt, h * HD:(h + 1) * HD],
                                     st_sb[:, kt, :],
                                     start=(kt == 0), stop=(kt == 1),
                                     skip_group_check=True)
            nc.vector.tensor_mul(ao[0:HD, pair, :], pv_ps[0:HD, :],
                                 rbcs[0][0:HD, :])
            nc.vector.tensor_mul(ao[HD:128, pair, :], pv_ps[HD:128, :],
                                 rbcs[1][HD:128, :])
        # out-proj + bias + residual
        for ot in range(IC):
            wt = wpool.tile([128, IC, 128], BF16, tag="wsl")
            nc.sync.dma_start(out=wt, in_=w_out[ot])
            ps = aps_mm.tile([128, T], F32, tag="mm")
            for ic in range(IC):
                nc.tensor.matmul(ps, wt[:, ic, :], ao[:, ic, :],
                                 start=(ic == 0), stop=(ic == IC - 1))
            nc.vector.scalar_tensor_tensor(new_resid[:, ot, :], in0=ps,
                                           scalar=b_out[:, ot:ot + 1],
                                           in1=old_resid[:, ot, :],
                                           op0=ALU.add, op1=ALU.add)

    def gate(s, tokens_fm, w1sb, w2sb, bsb, ctx):
        """ptm[:, 2s:2s+2, :] = softmax_E(tokens.W1 + ctx.W2 + b); also pfm."""
        ct_ps = aps_sm.tile([1, E], F32, tag="sm")
        for ic in range(IC):
            nc.tensor.matmul(ct_ps, ctx[:, ic, :], w2sb[:, ic, :],
                             start=(ic == 0), stop=(ic == IC - 1))
        crow = awork.tile([1, E], F32, tag="crow")
        nc.vector.tensor_add(crow, ct_ps, bsb)
        crow_bc = awork.tile([128, E], F32, tag="crow_bc")
        nc.gpsimd.partition_broadcast(crow_bc, crow)
        for tt in range(2):
            lg_ps = aps_tr.tile([128, E], F32, tag="tr")
            for ic in range(IC):
                nc.tensor.matmul(lg_ps, tokens_fm[:, ic, tt * 128:(tt + 1) * 128],
                                 w1sb[:, ic, :],
                                 start=(ic == 0), stop=(ic == IC - 1))
            lg = awork.tile([128, E], F32, tag="lg")
            nc.vector.tensor_add(lg, lg_ps, crow_bc)
            nm = awork.tile([128, 1], F32, tag="gnm")
            nc.vector.reduce_max(nm, lg, axis=AX.X, negate=True)
            gs = awork.tile([128, 1], F32, tag="gs")
            nc.scalar.activation(ptm[:, 2 * s + tt, :], lg, AF.Exp, bias=nm,
                                 accum_out=gs)
            gr = awork.tile([128, 1], F32, tag="gr")
            nc.vector.reciprocal(gr, gs)
            nc.vector.tensor_scalar_mul(ptm[:, 2 * s + tt, :],
                                        ptm[:, 2 * s + tt, :], gr)
            tp = aps_tr.tile([E, 128], F32, tag="tr")
            nc.tensor.transpose(tp, ptm[:, 2 * s + tt, :], ident)
            nc.vector.tensor_copy(pfm[:, 2 * s + tt, :], tp)

    # ---- phase-1 body ---------------------------------------------------
    xq0 = aw1.tile([128, IC, T], F32R, tag="xq0")
    nc.sync.dma_start(out=xq0, in_=dt["xq"])
    xt0 = aw1.tile([128, IC, T], F32R, tag="xt0")
    nc.sync.dma_start(out=xt0, in_=dt["xt"])
    xib = aw1.tile([128, IC, T], BF16, tag="xib")
    nc.vector.tensor_copy(xib, xi0)

    qn = aw1.tile([128, IC, T], BF16, tag="qn")
    ln_fm(qn, xq0, lnp["g_lnq"], lnp["b_lnq"])
    q1 = aw1.tile([128, IC, T], F32R, tag="q1")
    attention(q1, xq0, qn, qn, dt["w_saqk"], dt["wv_sa"],
              b_sa.rearrange("p a b -> p (a b)"), dt["w_sao"], b_sao)

    qn2 = aw1.tile([128, IC, T], BF16, tag="qn2")
    ln_fm(qn2, q1, lnp["g_lnc"], lnp["b_lnc"])
    attention(q2, q1, qn2, xib, dt["w_caqk"], dt["wv_ca"],
              b_ca.rearrange("p a b -> p (a b)"), dt["w_cao"], b_cao)

    # contexts: mean over tokens
    ictx = awork.tile([128, IC, 1], F32R, tag="ictx")
    tctx = awork.tile([128, IC, 1], F32R, tag="tctx")
    with nc.allow_low_precision(reason="f32r shares f32 bits; DVE sum is fp32"):
        for ic in range(IC):
            nc.vector.reduce_sum(ictx[:, ic, :], xi0[:, ic, :], axis=AX.X)
            nc.vector.reduce_sum(tctx[:, ic, :], xt0[:, ic, :], axis=AX.X)
    nc.scalar.mul(ictx.rearrange("p a b -> p (a b)"),
                  ictx.rearrange("p a b -> p (a b)"), 1.0 / T)
    nc.scalar.mul(tctx.rearrange("p a b -> p (a b)"),
                  tctx.rearrange("p a b -> p (a b)"), 1.0 / T)

    # routers: query stream uses txt gate on q2; image stream uses img gate
    gate(0, q2, w_tg1, w_tg2, b_tg, ictx)
    gate(1, xi0, w_ig1, w_ig2, b_ig, tctx)
    # probs / 256 compensates the x256 fp8 weight scale of w2
    nc.scalar.mul(ps8.rearrange("p a b -> p (a b)"),
                  ptm.rearrange("p a b -> p (a b)"), 1.0 / WS)

    # moe input (fp8): [ lnf(q2) ; xi0 ]
    lnf_scr = aw1.tile([128, IC, T], F32, tag="lnf_scr")
    ln_fm(lnf_scr, q2, lnp["g_lnf"], lnp["b_lnf"], dst8=x8[:, :, 0:T])
    with nc.allow_low_precision(reason="fp8 MoE inputs by design"):
        nc.vector.tensor_copy(x8[:, :, T:T2], xi0)

    # moe accumulator (token-major) init: residual + sum_e probs_e * b2_e
    b2mr = awork.tile([E, H], F32R, tag="b2mr")
    nc.vector.tensor_copy(b2mr, b2m)
    pfmr = awork.tile([E, NTT, 128], F32R, tag="pfmr")
    nc.vector.tensor_copy(pfmr, pfm)
    for tt in range(NTT):
        src = q2 if tt < 2 else xi0
        t0 = (tt % 2) * 128
        b2sb = awork.tile([128, H], F32, tag="b2sb")
        for oh in range(2):
            b2ps = aps_mm.tile([128, 512], F32, tag="mm", name=f"b2ps_{tt}_{oh}")
            nc.tensor.matmul(b2ps, pfmr[:, tt, :],
                             b2mr[:, oh * 512:(oh + 1) * 512],
                             start=True, stop=True)
            nc.scalar.copy(b2sb[:, oh * 512:(oh + 1) * 512], b2ps)
        for oc in range(IC):
            tp = aps_tr.tile([128, 128], F32R, tag="tr")
            nc.tensor.transpose(tp, src[:, oc, t0:t0 + 128], identr)
            nc.vector.tensor_add(
                acc[:, tt, oc * 128:(oc + 1) * 128], tp.bitcast(F32),
                b2sb[:, oc * 128:(oc + 1) * 128])

    for p in (wpool, aw1, awork, aps_sm, aps_pv, aps_tr, aps_mm):
        p.release()

    # ====================================================================
    # phase 2: dense fp8 DoubleRow MoE over both streams (512 tokens)
    # ====================================================================
    mps_h = tc.alloc_tile_pool(name="mps_h", bufs=2, space="PSUM")
    mps_o = tc.alloc_tile_pool(name="mps_o", bufs=4, space="PSUM")
    hpool = tc.alloc_tile_pool(name="hpool", bufs=2)
    mw1 = tc.alloc_tile_pool(name="mw1", bufs=5)
    mw2 = tc.alloc_tile_pool(name="mw2", bufs=18)

    for e in range(E):
        # GEMM1: h = gelu(x @ W1 / 256 + b1), f-major fp8 [128, FT, T2]
        w1t = []
        for k in range(KT1):
            t = mw1.tile([128, 2, F], FP8, tag="w1sl")
            nc.sync.dma_start(out=t, in_=dt["w1"][e, k])
            w1t.append(t)
        h8 = hpool.tile([128, FT, T2], FP8, tag="h8")
        for ft in range(FT):
            hps = mps_h.tile([128, T2], F32, tag="h")
            for k in range(KT1):
                nc.tensor.matmul(hps, w1t[k][:, :, ft * 128:(ft + 1) * 128],
                                 x8[:, 2 * k:2 * k + 2, :],
                                 start=(k == 0), stop=(k == KT1 - 1),
                                 perf_mode=DR)
            nc.scalar.activation(h8[:, ft, :], hps, AF.Gelu,
                                 bias=b1f[:, e, ft:ft + 1], scale=1.0 / WS)
        # GEMM2: o_tm = h.T @ W2 (h stationary -> token-major out);
        # acc += probs/256 * o
        w2t = []
        for k in range(KT2):
            t = mw2.tile([128, 2, H], FP8, tag="w2sl")
            nc.sync.dma_start(out=t, in_=dt["w2"][e, k])
            w2t.append(t)
        for tt in range(NTT):
            ops_ = [mps_o.tile([128, 512], F32, tag="o", name=f"o_{e}_{tt}_{oh}")
                    for oh in range(2)]
            for k in range(KT2):
                hslice = h8[:, 2 * k:2 * k + 2, tt * 128:(tt + 1) * 128]
                for oh in range(2):
                    nc.tensor.matmul(ops_[oh], hslice,
                                     w2t[k][:, :, oh * 512:(oh + 1) * 512],
                                     start=(k == 0), stop=(k == KT2 - 1),
                                     perf_mode=DR)
            for oh in range(2):
                nc.vector.scalar_tensor_tensor(
                    acc[:, tt, oh * 512:(oh + 1) * 512], in0=ops_[oh],
                    scalar=ps8[:, tt, e:e + 1],
                    in1=acc[:, tt, oh * 512:(oh + 1) * 512],
                    op0=ALU.mult, op1=ALU.add)

    # ---- outputs: already token-major ----------------------------------
    nc.sync.dma_start(out=dt["oq"], in_=acc[:, 0:2, :])
    nc.sync.dma_start(out=dt["oi"], in_=acc[:, 2:4, :])

    for p in (mw2, mw1, hpool, mps_o, mps_h, persist):
        p.release()


# ----------------------------------------------------------------------------
# host-side prep + run
# ----------------------------------------------------------------------------

_NC = None
LAST_EXEC_NS = None


def _get_nc():
    global _NC
    if _NC is None:
        _NC = _build_program()
    return _NC


def _prep_inputs(inp):
    """Build the per-core in_maps from the full (unsharded) numpy inputs."""
    f = np.float32

    def c(a):
        return np.ascontiguousarray(a, dtype=f)

    def q8(a):  # scale + quantize to TRN e4m3 (clip to TRN e4m3 max normal)
        return np.ascontiguousarray(
            np.clip(np.asarray(a, np.float32) * WS, -240, 240).astype(E4NP))

    def bf(a):
        return np.ascontiguousarray(np.asarray(a, np.float32).astype(BFNP))

    shared = {}
    for p, inw, inb, outw, outb in (("sa", "sa_in_w", "sa_in_b", "sa_out_w", "sa_out_b"),
                                    ("ca", "ca_in_w", "ca_in_b", "ca_out_w", "ca_out_b")):
        w3 = np.asarray(inp[inw], np.float32)      # [3H, H] packed q,k,v
        shared[f"w_{p}qk"] = bf(w3[:2 * H].reshape(2, 8, 128, IC, 128).transpose(0, 1, 4, 3, 2))
        shared[f"wv_{p}"] = bf(w3[2 * H:].T.reshape(IC, 128, H).transpose(1, 0, 2))
        shared[f"b_{p}"] = c(inp[inb].reshape(3, IC, 128).transpose(2, 0, 1))
        shared[f"w_{p}o"] = bf(inp[outw].reshape(8, 128, IC, 128).transpose(0, 3, 2, 1))
        # fold the V bias through the out-proj: out_w @ bv + out_b
        b_eff = np.asarray(inp[outb], np.float32) + \
            np.asarray(inp[outw], np.float32) @ np.asarray(inp[inb], np.float32)[2 * H:]
        shared[f"b_{p}o"] = c(b_eff.reshape(IC, 128).T)
    shared["w_ig1"] = c(inp["img_gate_w"][:, :H].T.reshape(IC, 128, E).transpose(1, 0, 2))
    shared["w_ig2"] = c(inp["img_gate_w"][:, H:].T.reshape(IC, 128, E).transpose(1, 0, 2))
    shared["b_ig"] = c(inp["img_gate_b"][None, :])
    shared["w_tg1"] = c(inp["txt_gate_w"][:, :H].T.reshape(IC, 128, E).transpose(1, 0, 2))
    shared["w_tg2"] = c(inp["txt_gate_w"][:, H:].T.reshape(IC, 128, E).transpose(1, 0, 2))
    shared["b_tg"] = c(inp["txt_gate_b"][None, :])
    for n, k in (("g_lnq", "lnq_g"), ("b_lnq", "lnq_b"), ("g_lnc", "lnc_g"),
                 ("b_lnc", "lnc_b"), ("g_lnf", "lnf_g"), ("b_lnf", "lnf_b")):
        shared[n] = c(inp[k].reshape(IC, 128).T)
    # fp8 DoubleRow pair layout: [e, k, part(128), pair, out]
    shared["w1"] = q8(inp["ew1"].reshape(E, KT1, 2, 128, F).transpose(0, 1, 3, 2, 4))
    shared["b1"] = c(inp["eb1"].reshape(E, FT, 128).transpose(2, 0, 1))
    shared["w2"] = q8(inp["ew2"].reshape(E, KT2, 2, 128, H).transpose(0, 1, 3, 2, 4))
    shared["b2"] = c(inp["eb2"])

    def fm(a):  # [T, H] -> [128, IC, T]
        return c(a.T.reshape(IC, 128, T).transpose(1, 0, 2))

    in_maps = []
    for b in range(B):
        m = dict(shared)
        m["xq"] = fm(np.asarray(inp["query_tokens"][b]))
        m["xi"] = fm(np.asarray(inp["image_tokens"][b]))
        m["xt"] = fm(np.asarray(inp["text_context"][b]))
        in_maps.append(m)
    return in_maps


def _run(inp, trace=False):
    global LAST_EXEC_NS
    nc = _get_nc()
    in_maps = _prep_inputs(inp)
    res = run_bass_kernel_spmd(nc, in_maps, core_ids=list(range(B)), trace=trace)
    LAST_EXEC_NS = res.exec_time_ns
    oq = np.empty((B, T, H), np.float32)
    oi = np.empty((B, T, H), np.float32)
    for b in range(B):
        oq[b] = res.results[b]["oq"].transpose(1, 0, 2).reshape(T, H)
        oi[b] = res.results[b]["oi"].transpose(1, 0, 2).reshape(T, H)
    return oq, oi


def kernel(**inputs):
    return _run(inputs, trace=False)


# revision 26
# speedup vs baseline: 1.9378x; 1.0327x over previous
"""CrossModalMoELayer Trainium2 Bass kernel.

Sharding: data-parallel over batch B=8 across the 8 NeuronCores (one batch
element per core). Each core runs the full layer for its batch element:
self-attention, cross-attention, gating, and the dense 8-expert MoE.

Attention/gating run in float32r (full-rate fp32 PE mode). The MoE - the
dominant compute - runs in fp8 (e4m3) with DoubleRow perf mode (2 fp8
MACs/cell/cycle, ~2x PE throughput). Expert weights are host-quantized to
e4m3 with a x256 scale (keeps the 0.02-scale weights out of the subnormal
range); the scale is folded back out in the gelu (scale=1/256) and the
router-prob accumulate (probs/256).

Layouts on device:
  feature-major ("fm"): [feat_part=128, feat_chunk, tokens]  - activations
  MoE output accumulates token-major: [tok_part=128, tok_tile, feature] so
  router probs apply as native per-partition scalars and the final store
  needs no transposes.

kernel(**inputs) takes the FULL unsharded inputs (numpy, keyed as in
setup_inputs()) and returns the full (query_tokens, image_tokens) tuple.
"""

import numpy as np
import ml_dtypes

import concourse.bass as bass
import concourse.tile as tile
from concourse import bacc, mybir
from concourse.bass_utils import run_bass_kernel_spmd
from concourse.masks import make_identity

B, T, H, NH, HD, F, E = 8, 256, 1024, 16, 64, 4096, 8
IC = H // 128          # 8 feature chunks of the model dim
FT = F // 128          # 32 feature chunks of the FFN dim
KT1 = IC // 2          # 4 DoubleRow k-tiles for GEMM1 (contraction H)
KT2 = FT // 2          # 16 DoubleRow k-tiles for GEMM2 (contraction F)
NTT = 4                # token tiles of 128 across both streams
T2 = 2 * T             # query tokens + image tokens concatenated
EPS = 1e-5
WS = 256.0             # fp8 weight scale

F32 = mybir.dt.float32
F32R = mybir.dt.float32r
BF16 = mybir.dt.bfloat16
FP8 = mybir.dt.float8e4
E4NP = ml_dtypes.float8_e4m3
BFNP = ml_dtypes.bfloat16
AX = mybir.AxisListType
ALU = mybir.AluOpType
AF = mybir.ActivationFunctionType
DR = mybir.MatmulPerfMode.DoubleRow


# ----------------------------------------------------------------------------
# program builder
# ----------------------------------------------------------------------------

def _build_program():
    nc = bacc.Bacc(
        "TRN2",
        target_bir_lowering=False,
        debug=False,
        enable_asserts=False,
        num_devices=8,
    )

    dt = {}

    def din(name, shape, d=F32):
        dt[name] = nc.dram_tensor(name, list(shape), d, kind="ExternalInput").ap()
        return dt[name]

    def dout(name, shape):
        dt[name] = nc.dram_tensor(name, list(shape), F32, kind="ExternalOutput").ap()
        return dt[name]

    # activations (per core)
    din("xq", (128, IC, T), F32R)
    din("xi", (128, IC, T), F32R)
    din("xt", (128, IC, T), F32R)
    # attention weights (bf16): q/k proj [proj, ot, i(128), ic, o(128)],
    # v proj token-major moving [i(128), ic, d], out-proj [ot, i, ic, o]
    din("w_saqk", (2, 8, 128, IC, 128), BF16)
    din("wv_sa", (128, IC, H), BF16)
    din("b_sa", (128, 3, IC))
    din("w_sao", (8, 128, IC, 128), BF16)
    din("b_sao", (128, IC))
    din("w_caqk", (2, 8, 128, IC, 128), BF16)
    din("wv_ca", (128, IC, H), BF16)
    din("b_ca", (128, 3, IC))
    din("w_cao", (8, 128, IC, 128), BF16)
    din("b_cao", (128, IC))
    # gates
    din("w_ig1", (128, IC, E), F32R)
    din("w_ig2", (128, IC, E), F32R)
    din("b_ig", (1, E))
    din("w_tg1", (128, IC, E), F32R)
    din("w_tg2", (128, IC, E), F32R)
    din("b_tg", (1, E))
    # layernorms [128, IC]
    for n in ("g_lnq", "b_lnq", "g_lnc", "b_lnc", "g_lnf", "b_lnf"):
        din(n, (128, IC))
    # experts (fp8, x256-scaled, DoubleRow pair layout)
    din("w1", (E, KT1, 128, 2, F), FP8)        # [e, k, i(128), pair, f]
    din("b1", (128, E, FT))
    din("w2", (E, KT2, 128, 2, H), FP8)        # [e, k, f(128), pair, o]
    din("b2", (E, H))
    # outputs token-major [t(128), tt, o]
    dout("oq", (128, 2, H))
    dout("oi", (128, 2, H))

    with tile.TileContext(nc) as tc:
        _trace_kernel(nc, tc, dt)

    nc.compile()
    return nc


def _trace_kernel(nc, tc, dt):
    persist = tc.alloc_tile_pool(name="persist", bufs=1)

    # ---- constants + small params --------------------------------------
    ident = persist.tile([128, 128], F32, tag="ident")
    make_identity(nc, ident)
    ones_f = persist.tile([128, 1], F32, tag="ones_f")
    nc.vector.memset(ones_f, 1.0)
    ones = persist.tile([128, 1], F32R, tag="ones")
    nc.vector.tensor_copy(ones, ones_f)
    ones_b = persist.tile([128, 1], BF16, tag="ones_b")
    nc.vector.tensor_copy(ones_b, ones_f)
    identr = persist.tile([128, 128], F32R, tag="identr")
    nc.vector.tensor_copy(identr, ident)
    eps_t = persist.tile([1, 1], F32, tag="eps")
    nc.vector.memset(eps_t, EPS)

    def load(name, shape, d=F32, pool=persist):
        t = pool.tile(list(shape), d, tag=f"ld_{name}")
        nc.sync.dma_start(out=t, in_=dt[name])
        return t

    xi0 = load("xi", (128, IC, T), F32R)
    b_sa = load("b_sa", (128, 3, IC))
    b_sao = load("b_sao", (128, IC))
    b_ca = load("b_ca", (128, 3, IC))
    b_cao = load("b_cao", (128, IC))
    w_ig1 = load("w_ig1", (128, IC, E), F32R)
    w_ig2 = load("w_ig2", (128, IC, E), F32R)
    b_ig = load("b_ig", (1, E))
    w_tg1 = load("w_tg1", (128, IC, E), F32R)
    w_tg2 = load("w_tg2", (128, IC, E), F32R)
    b_tg = load("b_tg", (1, E))
    lnp = {n: load(n, (128, IC)) for n in
           ("g_lnq", "b_lnq", "g_lnc", "b_lnc", "g_lnf", "b_lnf")}
    b1f = load("b1", (128, E, FT))
    b2m = load("b2", (E, H))

    # persistent activations
    q2 = persist.tile([128, IC, T], F32R, tag="q2")          # query after CA
    x8 = persist.tile([128, IC, T2], FP8, tag="x8")          # fp8 [lnf(q2); xi0]
    ptm = persist.tile([128, NTT, E], F32, tag="ptm")        # router probs (tm)
    ps8 = persist.tile([128, NTT, E], F32, tag="ps8")        # probs / 256
    pfm = persist.tile([E, NTT, 128], F32, tag="pfm")        # probs (fm, for b2)
    acc = persist.tile([128, NTT, H], F32, tag="acc")        # MoE accum (tm)

    # w1 weight pool outlives phase 1 so expert-0/1 weights prefetch
    # during attention instead of stalling the phase boundary
    mw1 = tc.alloc_tile_pool(name="mw1", bufs=5)

    # ====================================================================
    # phase 1: attention + gating + lnf (own pools, released before MoE)
    # ====================================================================
    aps_mm = tc.alloc_tile_pool(name="aps_mm", bufs=2, space="PSUM")
    aps_tr = tc.alloc_tile_pool(name="aps_tr", bufs=3, space="PSUM")
    aps_pv = tc.alloc_tile_pool(name="aps_pv", bufs=2, space="PSUM")
    aps_sm = tc.alloc_tile_pool(name="aps_sm", bufs=1, space="PSUM")
    awork = tc.alloc_tile_pool(name="awork", bufs=2)
    aw1 = tc.alloc_tile_pool(name="aw1", bufs=1)
    wpool = tc.alloc_tile_pool(name="wpool", bufs=4)

    def ln_fm(dst, src, g, b, dst8=None):
        """dst[:, ic, :] = LN over features of src (fm layout [128, IC, T]).

        If dst8 is given, the normalized result is additionally written to
        dst8 (fp8) with the fp32 intermediate kept in dst.
        """
        ntok = src.shape[2]
        sum_ps = aps_sm.tile([1, ntok], F32, tag="sm")
        for ic in range(IC):
            nc.tensor.matmul(sum_ps, ones, src[:, ic, :],
                             start=(ic == 0), stop=(ic == IC - 1))
        mean = awork.tile([1, ntok], F32, tag="ln_mean")
        nc.scalar.mul(mean, sum_ps, 1.0 / H)
        sumsq_ps = aps_sm.tile([1, ntok], F32, tag="sm")
        for ic in range(IC):
            xsq = awork.tile([128, ntok], F32R, tag="ln_xsq")
            nc.scalar.activation(xsq, src[:, ic, :], AF.Square)
            nc.tensor.matmul(sumsq_ps, ones, xsq,
                             start=(ic == 0), stop=(ic == IC - 1))
        msq = awork.tile([1, ntok], F32, tag="ln_msq")
        nc.vector.tensor_mul(msq, mean, mean)
        var = awork.tile([1, ntok], F32, tag="ln_var")
        nc.vector.scalar_tensor_tensor(var, in0=sumsq_ps, scalar=1.0 / H,
                                       in1=msq, op0=ALU.mult, op1=ALU.subtract)
        std = awork.tile([1, ntok], F32, tag="ln_std")
        nc.scalar.activation(std, var, AF.Sqrt, bias=eps_t)
        rstd = awork.tile([1, ntok], F32, tag="ln_rstd")
        nc.vector.reciprocal(rstd, std)
        negc = awork.tile([1, ntok], F32, tag="ln_negc")
        nc.vector.scalar_tensor_tensor(negc, in0=mean, scalar=-1.0,
                                       in1=rstd, op0=ALU.mult, op1=ALU.mult)
        a_bc = awork.tile([128, ntok], F32, tag="ln_abc")
        nc.gpsimd.partition_broadcast(a_bc, rstd)
        c_bc = awork.tile([128, ntok], F32, tag="ln_cbc")
        nc.gpsimd.partition_broadcast(c_bc, negc)
        for ic in range(IC):
            nc.vector.tensor_mul(dst[:, ic, :], src[:, ic, :], a_bc)
            nc.vector.tensor_add(dst[:, ic, :], dst[:, ic, :], c_bc)
            out_ic = dst[:, ic, :] if dst8 is None else dst8[:, ic, :]
            nc.vector.tensor_scalar(out=out_ic, in0=dst[:, ic, :],
                                    scalar1=g[:, ic:ic + 1], scalar2=b[:, ic:ic + 1],
                                    op0=ALU.mult, op1=ALU.add)

    def attention(new_resid, old_resid, qsrc, kvsrc, w_qk, wv, b_in, w_out,
                  b_out):
        """new_resid = old_resid + out_proj(MHA(q=qsrc, kv=kvsrc)).

        qsrc/kvsrc are fm bf16.  Scores are computed k-major (st[k, q]) so
        softmax needs no PE transposes: exp without max-subtraction (score
        range is ±4 here), per-q sums via ones-matmul, and PV emits the
        attention output directly feature-major with 1/sum folded in after.
        V is projected token-major (x stationary, Wv^T moving); the V bias
        is folded into b_out host-side (softmax weights sum to 1).
        """
        qf = aw1.tile([128, IC, T], BF16, tag="qf")
        kf = aw1.tile([128, IC, T], BF16, tag="kf")
        for proj, dst, src in ((0, qf, qsrc), (1, kf, kvsrc)):
            for ot in range(IC):
                wt = wpool.tile([128, IC, 128], BF16, tag="wsl")
                nc.sync.dma_start(out=wt, in_=w_qk[proj, ot])
                ps = aps_mm.tile([128, T], F32, tag="mm")
                for ic in range(IC):
                    nc.tensor.matmul(ps, wt[:, ic, :], src[:, ic, :],
                                     start=(ic == 0), stop=(ic == IC - 1))
                nc.scalar.add(dst[:, ot, :], ps,
                              b_in[:, proj * IC + ot:proj * IC + ot + 1])
        # V projection, token-major: vht[t, kt, d]
        vht = aw1.tile([128, 2, H], BF16, tag="vht")
        for dh in range(2):
            wvt = wpool.tile([128, IC, 512], BF16, tag="wvt", bufs=1)
            nc.sync.dma_start(out=wvt, in_=wv[:, :, dh * 512:(dh + 1) * 512])
            for kt in range(2):
                ps = aps_mm.tile([128, 512], F32, tag="mm")
                for ic in range(IC):
                    nc.tensor.matmul(ps, kvsrc[:, ic, kt * 128:(kt + 1) * 128],
                                     wvt[:, ic, :],
                                     start=(ic == 0), stop=(ic == IC - 1))
                nc.scalar.copy(vht[:, kt, dh * 512:(dh + 1) * 512], ps)
        # attention per head-pair; output directly feature-major
        ao = aw1.tile([128, IC, T], BF16, tag="ao")
        for pair in range(NH // 2):
            pv_ps = aps_pv.tile([128, T], F32, tag="pv")
            rbcs = []
            for j in range(2):
                h = 2 * pair + j
                b0 = j * HD
                st_sb = awork.tile([128, 2, T], BF16, tag="st", name=f"st_{h}", bufs=3)
                sum_ps = aps_sm.tile([1, T], F32, tag="sm")
                for kt in range(2):
                    st_ps = aps_tr.tile([128, T], F32, tag="tr")
                    nc.tensor.matmul(st_ps,
                                     kf[b0:b0 + HD, pair, kt * 128:(kt + 1) * 128],
                                     qf[b0:b0 + HD, pair, :],
                                     start=True, stop=True)
                    nc.scalar.activation(st_sb[:, kt, :], st_ps, AF.Exp,
                                         scale=0.125)
                    nc.tensor.matmul(sum_ps, ones_b, st_sb[:, kt, :],
                                     start=(kt == 0), stop=(kt == 1))
                rs = awork.tile([1, T], F32, tag="rs")
                nc.vector.reciprocal(rs, sum_ps)
                rbc = awork.tile([128, T], F32, tag="rbc",
                                 name=f"rbc_{pair}_{j}", bufs=3)
                nc.gpsimd.partition_broadcast(rbc, rs)
                rbcs.append(rbc)
                for kt in range(2):
                    nc.tensor.matmul(pv_ps[b0:b0 + HD, :],
                                     vht[:, kt, h * HD:(h + 1) * HD],
                                     st_sb[:, kt, :],
                                     start=(kt == 0), stop=(kt == 1),
                                     skip_group_check=True)
            nc.vector.tensor_mul(ao[0:HD, pair, :], pv_ps[0:HD, :],
                                 rbcs[0][0:HD, :])
            nc.vector.tensor_mul(ao[HD:128, pair, :], pv_ps[HD:128, :],
                                 rbcs[1][HD:128, :])
        # out-proj + bias + residual
        for ot in range(IC):
            wt = wpool.tile([128, IC, 128], BF16, tag="wsl")
            nc.sync.dma_start(out=wt, in_=w_out[ot])
            ps = aps_mm.tile([128, T], F32, tag="mm")
            for ic in range(IC):
                nc.tensor.matmul(ps, wt[:, ic, :], ao[:, ic, :],
                                 start=(ic == 0), stop=(ic == IC - 1))
            nc.vector.scalar_tensor_tensor(new_resid[:, ot, :], in0=ps,
                                           scalar=b_out[:, ot:ot + 1],
                                           in1=old_resid[:, ot, :],
                                           op0=ALU.add, op1=ALU.add)

    def gate(s, tokens_fm, w1sb, w2sb, bsb, ctx):
        """ptm[:, 2s:2s+2, :] = softmax_E(tokens.W1 + ctx.W2 + b); also pfm."""
        ct_ps = aps_sm.tile([1, E], F32, tag="sm")
        for ic in range(IC):
            nc.tensor.matmul(ct_ps, ctx[:, ic, :], w2sb[:, ic, :],
                             start=(ic == 0), stop=(ic == IC - 1))
        crow = awork.tile([1, E], F32, tag="crow")
        nc.vector.tensor_add(crow, ct_ps, bsb)
        crow_bc = awork.tile([128, E], F32, tag="crow_bc")
        nc.gpsimd.partition_broadcast(crow_bc, crow)
        for tt in range(2):
            lg_ps = aps_tr.tile([128, E], F32, tag="tr")
            for ic in range(IC):
                nc.tensor.matmul(lg_ps, tokens_fm[:, ic, tt * 128:(tt + 1) * 128],
                                 w1sb[:, ic, :],
                                 start=(ic == 0), stop=(ic == IC - 1))
            lg = awork.tile([128, E], F32, tag="lg")
            nc.vector.tensor_add(lg, lg_ps, crow_bc)
            nm = awork.tile([128, 1], F32, tag="gnm")
            nc.vector.reduce_max(nm, lg, axis=AX.X, negate=True)
            gs = awork.tile([128, 1], F32, tag="gs")
            nc.scalar.activation(ptm[:, 2 * s + tt, :], lg, AF.Exp, bias=nm,
                                 accum_out=gs)
            gr = awork.tile([128, 1], F32, tag="gr")
            nc.vector.reciprocal(gr, gs)
            nc.vector.tensor_scalar_mul(ptm[:, 2 * s + tt, :],
                                        ptm[:, 2 * s + tt, :], gr)
            tp = aps_tr.tile([E, 128], F32, tag="tr")
            nc.tensor.transpose(tp, ptm[:, 2 * s + tt, :], ident)
            nc.vector.tensor_copy(pfm[:, 2 * s + tt, :], tp)

    # ---- phase-1 body ---------------------------------------------------
    xq0 = aw1.tile([128, IC, T], F32R, tag="xq0")
    nc.sync.dma_start(out=xq0, in_=dt["xq"])
    xt0 = aw1.tile([128, IC, T], F32R, tag="xt0")
    nc.sync.dma_start(out=xt0, in_=dt["xt"])
    xib = aw1.tile([128, IC, T], BF16, tag="xib")
    nc.vector.tensor_copy(xib, xi0)

    b2mr = awork.tile([E, H], F32R, tag="b2mr", bufs=1)
    nc.vector.tensor_copy(b2mr, b2m)

    def acc_init(tt, src, pfmr):
        """acc[:, tt, :] = src_tm + sum_e probs_e * b2_e for token tile tt."""
        t0 = (tt % 2) * 128
        b2sb = awork.tile([128, H], F32, tag="b2sb", bufs=2)
        for oh in range(2):
            b2ps = aps_mm.tile([128, 512], F32, tag="mm", name=f"b2ps_{tt}_{oh}")
            nc.tensor.matmul(b2ps, pfmr[:, tt, :],
                             b2mr[:, oh * 512:(oh + 1) * 512],
                             start=True, stop=True)
            nc.scalar.copy(b2sb[:, oh * 512:(oh + 1) * 512], b2ps)
        for oc in range(IC):
            tp = aps_tr.tile([128, 128], F32R, tag="tr")
            nc.tensor.transpose(tp, src[:, oc, t0:t0 + 128], identr)
            nc.vector.tensor_add(
                acc[:, tt, oc * 128:(oc + 1) * 128], tp.bitcast(F32),
                b2sb[:, oc * 128:(oc + 1) * 128])

    # ---- image-stream prep (independent of attention, runs first) -------
    ictx = awork.tile([128, IC, 1], F32R, tag="ictx")
    tctx = awork.tile([128, IC, 1], F32R, tag="tctx")
    with nc.allow_low_precision(reason="f32r shares f32 bits; DVE sum is fp32"):
        for ic in range(IC):
            nc.vector.reduce_sum(ictx[:, ic, :], xi0[:, ic, :], axis=AX.X)
            nc.vector.reduce_sum(tctx[:, ic, :], xt0[:, ic, :], axis=AX.X)
    nc.scalar.mul(ictx.rearrange("p a b -> p (a b)"),
                  ictx.rearrange("p a b -> p (a b)"), 1.0 / T)
    nc.scalar.mul(tctx.rearrange("p a b -> p (a b)"),
                  tctx.rearrange("p a b -> p (a b)"), 1.0 / T)
    gate(1, xi0, w_ig1, w_ig2, b_ig, tctx)
    with nc.allow_low_precision(reason="fp8 MoE inputs by design"):
        nc.vector.tensor_copy(x8[:, :, T:T2], xi0)
    pfmr = awork.tile([E, NTT, 128], F32R, tag="pfmr", bufs=1)
    nc.vector.tensor_copy(pfmr[:, 2:4, :], pfm[:, 2:4, :])
    acc_init(2, xi0, pfmr)
    acc_init(3, xi0, pfmr)

    # ---- attention chain ------------------------------------------------
    qn = aw1.tile([128, IC, T], BF16, tag="qn")
    ln_fm(qn, xq0, lnp["g_lnq"], lnp["b_lnq"])
    q1 = aw1.tile([128, IC, T], F32R, tag="q1")
    attention(q1, xq0, qn, qn, dt["w_saqk"], dt["wv_sa"],
              b_sa.rearrange("p a b -> p (a b)"), dt["w_sao"], b_sao)

    qn2 = aw1.tile([128, IC, T], BF16, tag="qn2")
    ln_fm(qn2, q1, lnp["g_lnc"], lnp["b_lnc"])
    attention(q2, q1, qn2, xib, dt["w_caqk"], dt["wv_ca"],
              b_ca.rearrange("p a b -> p (a b)"), dt["w_cao"], b_cao)

    # query-stream gate on q2, with image context
    gate(0, q2, w_tg1, w_tg2, b_tg, ictx)
    # probs / 256 compensates the x256 fp8 weight scale of w2
    nc.scalar.mul(ps8.rearrange("p a b -> p (a b)"),
                  ptm.rearrange("p a b -> p (a b)"), 1.0 / WS)

    # moe input (fp8): [ lnf(q2) ; xi0 ]
    lnf_scr = aw1.tile([128, IC, T], F32, tag="lnf_scr")
    ln_fm(lnf_scr, q2, lnp["g_lnf"], lnp["b_lnf"], dst8=x8[:, :, 0:T])

    nc.vector.tensor_copy(pfmr[:, 0:2, :], pfm[:, 0:2, :])
    acc_init(0, q2, pfmr)
    acc_init(1, q2, pfmr)

    for p in (wpool, aw1, awork, aps_sm, aps_pv, aps_tr, aps_mm):
        p.release()

    # ====================================================================
    # phase 2: dense fp8 DoubleRow MoE over both streams (512 tokens)
    # ====================================================================
    mps_h = tc.alloc_tile_pool(name="mps_h", bufs=2, space="PSUM")
    mps_o = tc.alloc_tile_pool(name="mps_o", bufs=4, space="PSUM")
    hpool = tc.alloc_tile_pool(name="hpool", bufs=2)
    mw2 = tc.alloc_tile_pool(name="mw2", bufs=18)

    for e in range(E):
        # GEMM1: h = gelu(x @ W1 / 256 + b1), f-major fp8 [128, FT, T2]
        w1t = []
        for k in range(KT1):
            t = mw1.tile([128, 2, F], FP8, tag="w1sl")
            nc.sync.dma_start(out=t, in_=dt["w1"][e, k])
            w1t.append(t)
        h8 = hpool.tile([128, FT, T2], FP8, tag="h8")
        for ft in range(FT):
            hps = mps_h.tile([128, T2], F32, tag="h")
            for k in range(KT1):
                nc.tensor.matmul(hps, w1t[k][:, :, ft * 128:(ft + 1) * 128],
                                 x8[:, 2 * k:2 * k + 2, :],
                                 start=(k == 0), stop=(k == KT1 - 1),
                                 perf_mode=DR)
            nc.scalar.activation(h8[:, ft, :], hps, AF.Gelu,
                                 bias=b1f[:, e, ft:ft + 1], scale=1.0 / WS)
        # GEMM2: o_tm = h.T @ W2 (h stationary -> token-major out);
        # acc += probs/256 * o
        w2t = []
        for k in range(KT2):
            t = mw2.tile([128, 2, H], FP8, tag="w2sl")
            nc.sync.dma_start(out=t, in_=dt["w2"][e, k])
            w2t.append(t)
        for tt in range(NTT):
            ops_ = [mps_o.tile([128, 512], F32, tag="o", name=f"o_{e}_{tt}_{oh}")
                    for oh in range(2)]
            for k in range(KT2):
                hslice = h8[:, 2 * k:2 * k + 2, tt * 128:(tt + 1) * 128]
                for oh in range(2):
                    nc.tensor.matmul(ops_[oh], hslice,
                                     w2t[k][:, :, oh * 512:(oh + 1) * 512],
                                     start=(k == 0), stop=(k == KT2 - 1),
                                     perf_mode=DR)
            for oh in range(2):
                nc.vector.scalar_tensor_tensor(
                    acc[:, tt, oh * 512:(oh + 1) * 512], in0=ops_[oh],
                    scalar=ps8[:, tt, e:e + 1],
                    in1=acc[:, tt, oh * 512:(oh + 1) * 512],
                    op0=ALU.mult, op1=ALU.add)

    # ---- outputs: already token-major ----------------------------------
    nc.sync.dma_start(out=dt["oq"], in_=acc[:, 0:2, :])
    nc.sync.dma_start(out=dt["oi"], in_=acc[:, 2:4, :])

    for p in (mw2, hpool, mps_o, mps_h, mw1, persist):
        p.release()


# ----------------------------------------------------------------------------
# host-side prep + run
# ----------------------------------------------------------------------------

_NC = None
LAST_EXEC_NS = None


def _get_nc():
    global _NC
    if _NC is None:
        _NC = _build_program()
    return _NC


def _prep_inputs(inp):
    """Build the per-core in_maps from the full (unsharded) numpy inputs."""
    f = np.float32

    def c(a):
        return np.ascontiguousarray(a, dtype=f)

    def q8(a):  # scale + quantize to TRN e4m3 (clip to TRN e4m3 max normal)
        return np.ascontiguousarray(
            np.clip(np.asarray(a, np.float32) * WS, -240, 240).astype(E4NP))

    def bf(a):
        return np.ascontiguousarray(np.asarray(a, np.float32).astype(BFNP))

    shared = {}
    for p, inw, inb, outw, outb in (("sa", "sa_in_w", "sa_in_b", "sa_out_w", "sa_out_b"),
                                    ("ca", "ca_in_w", "ca_in_b", "ca_out_w", "ca_out_b")):
        w3 = np.asarray(inp[inw], np.float32)      # [3H, H] packed q,k,v
        shared[f"w_{p}qk"] = bf(w3[:2 * H].reshape(2, 8, 128, IC, 128).transpose(0, 1, 4, 3, 2))
        shared[f"wv_{p}"] = bf(w3[2 * H:].T.reshape(IC, 128, H).transpose(1, 0, 2))
        shared[f"b_{p}"] = c(inp[inb].reshape(3, IC, 128).transpose(2, 0, 1))
        shared[f"w_{p}o"] = bf(inp[outw].reshape(8, 128, IC, 128).transpose(0, 3, 2, 1))
        # fold the V bias through the out-proj: out_w @ bv + out_b
        b_eff = np.asarray(inp[outb], np.float32) + \
            np.asarray(inp[outw], np.float32) @ np.asarray(inp[inb], np.float32)[2 * H:]
        shared[f"b_{p}o"] = c(b_eff.reshape(IC, 128).T)
    shared["w_ig1"] = c(inp["img_gate_w"][:, :H].T.reshape(IC, 128, E).transpose(1, 0, 2))
    shared["w_ig2"] = c(inp["img_gate_w"][:, H:].T.reshape(IC, 128, E).transpose(1, 0, 2))
    shared["b_ig"] = c(inp["img_gate_b"][None, :])
    shared["w_tg1"] = c(inp["txt_gate_w"][:, :H].T.reshape(IC, 128, E).transpose(1, 0, 2))
    shared["w_tg2"] = c(inp["txt_gate_w"][:, H:].T.reshape(IC, 128, E).transpose(1, 0, 2))
    shared["b_tg"] = c(inp["txt_gate_b"][None, :])
    for n, k in (("g_lnq", "lnq_g"), ("b_lnq", "lnq_b"), ("g_lnc", "lnc_g"),
                 ("b_lnc", "lnc_b"), ("g_lnf", "lnf_g"), ("b_lnf", "lnf_b")):
        shared[n] = c(inp[k].reshape(IC, 128).T)
    # fp8 DoubleRow pair layout: [e, k, part(128), pair, out]
    shared["w1"] = q8(inp["ew1"].reshape(E, KT1, 2, 128, F).transpose(0, 1, 3, 2, 4))
    shared["b1"] = c(inp["eb1"].reshape(E, FT, 128).transpose(2, 0, 1))
    shared["w2"] = q8(inp["ew2"].reshape(E, KT2, 2, 128, H).transpose(0, 1, 3, 2, 4))
    shared["b2"] = c(inp["eb2"])

    def fm(a):  # [T, H] -> [128, IC, T]
        return c(a.T.reshape(IC, 128, T).transpose(1, 0, 2))

    in_maps = []
    for b in range(B):
        m = dict(shared)
        m["xq"] = fm(np.asarray(inp["query_tokens"][b]))
        m["xi"] = fm(np.asarray(inp["image_tokens"][b]))
        m["xt"] = fm(np.asarray(inp["text_context"][b]))
        in_maps.append(m)
    return in_maps


def _run(inp, trace=False):
    global LAST_EXEC_NS
    nc = _get_nc()
    in_maps = _prep_inputs(inp)
    res = run_bass_kernel_spmd(nc, in_maps, core_ids=list(range(B)), trace=trace)
    LAST_EXEC_NS = res.exec_time_ns
    oq = np.empty((B, T, H), np.float32)
    oi = np.empty((B, T, H), np.float32)
    for b in range(B):
        oq[b] = res.results[b]["oq"].transpose(1, 0, 2).reshape(T, H)
        oi[b] = res.results[b]["oi"].transpose(1, 0, 2).reshape(T, H)
    return oq, oi


def kernel(**inputs):
    return _run(inputs, trace=False)
